# revision 1
# baseline (speedup 1.0000x reference)
# Causal self-attention on 8 NeuronCores (Trainium2, Bass/Tile).
#
# Problem: B=2, T=2048, C=1024, H=16 heads (hd=64).
#   qkv = x @ W_qkv + b_qkv ; per-head causal softmax attention ; y = att_out @ W_proj + b_proj
#
# Sharding: tensor-parallel over heads x data-parallel over batch.
#   core = b*4 + g   (b in {0,1} batch, g in {0..3} head group of 4 heads)
#   Each core: qkv projection for its 4 heads (W_qkv column shard),
#   attention for those heads, then a partial row-shard projection
#   y_partial^T = W_proj[g-rows]^T @ att_out^T.  Host sums the 4 partials
#   per batch and adds b_proj.
#
# Everything on-chip is kept in "transposed" (feature-on-partition) layout so
# no P-matrix transposes are needed:
#   xT [C, T] (built once via PE transpose) -> qT,kT [64, T] per head (natural
#   output of W-stationary matmul), v [T, 64] per head (natural output of
#   xT-stationary matmul, stored with an appended ones-column so the PV matmul
#   also produces the softmax denominator), S^T [k, q] chunks (softmax over the
#   partition dim k becomes: exp on ACT with per-partition key-padding bias,
#   denominator from the ones-column of V), out^T [65, q] accumulated in PSUM.
# Matmuls use float32r (TF32-like, full PE rate at N>=256) with fp32 accumulate.

import numpy as np

T = 2048
C = 1024
HL = 4          # heads per core
HD = 64
CL = HL * HD    # 256 local channels
P = 128
NEG = -1e9

_cache = {}


def _build_nc():
    import concourse.bass as bass
    import concourse.mybir as mybir
    import concourse.tile as tile
    from concourse import bacc
    from concourse.masks import make_identity
    from contextlib import ExitStack

    f32 = mybir.dt.float32
    f32r = mybir.dt.float32r
    ADD = mybir.AluOpType.add
    EXP = mybir.ActivationFunctionType.Exp

    nc = bacc.Bacc("TRN2", target_bir_lowering=False)
    x_d = nc.declare_dram_parameter("x", [T, C], f32r, isOutput=False)
    wqk_d = nc.declare_dram_parameter("wqk", [C, 2 * CL], f32r, isOutput=False)
    wv_d = nc.declare_dram_parameter("wv", [C, CL], f32r, isOutput=False)
    bqk_d = nc.declare_dram_parameter("bqk", [P, 4], f32, isOutput=False)
    bv_d = nc.declare_dram_parameter("bv", [1, CL], f32, isOutput=False)
    kbias_d = nc.declare_dram_parameter("kbias", [P, 16], f32, isOutput=False)
    wproj_d = nc.declare_dram_parameter("wproj", [CL, C], f32r, isOutput=False)
    yT_d = nc.declare_dram_parameter("yT", [C, T], f32, isOutput=True)

    NT = T // P       # 16 t-tiles of 128
    NCC = C // P      # 8 contraction chunks of 128
    NQ = T // 512     # 4 q-quads of 512

    with tile.TileContext(nc) as tc, ExitStack() as ctx:
        singles = ctx.enter_context(tc.tile_pool(name="singles", bufs=1))

        # persistent SBUF
        qkT = singles.tile([P, 4, T], f32r)         # rows: [q f0,q f1,k f0,k f1]
        vv = singles.tile([P, NT, HL, HD + 1], f32r)  # v + ones col per head
        AT = singles.tile([P, 2, T], f32r)          # attention out^T (c' x t)
        tri01 = singles.tile([P, P], f32)          # lower-tri 1.0 / 0.0
        kbias_sb = singles.tile([P, 16], f32)
        bqk_sb = singles.tile([P, 4], f32)
        bv_sb = singles.tile([P, HL, HD], f32)
        wproj_sb = singles.tile([P, 2, C], f32r)
        ident = singles.tile([P, P], f32)
        identr = singles.tile([P, P], f32r)

        make_identity(nc, ident)
        # tri01[k, q] = 1.0 where q >= k else 0.0
        nc.gpsimd.memset(tri01, 1.0)
        nc.gpsimd.affine_select(
            out=tri01,
            in_=tri01,
            compare_op=mybir.AluOpType.is_ge,
            fill=0.0,
            base=0,
            pattern=[[1, P]],
            channel_multiplier=-1,
        )

        # small/params on the gpsimd (SWDGE) queue so the big x loads on
        # the sync queue aren't delayed
        nc.gpsimd.dma_start(out=kbias_sb, in_=kbias_d[:])
        nc.gpsimd.dma_start(out=bqk_sb, in_=bqk_d[:])
        nc.gpsimd.dma_start(
            out=bv_sb,
            in_=bv_d[:].rearrange("o (h d) -> o h d", h=HL).to_broadcast([P, HL, HD]),
        )
        nc.vector.tensor_copy(out=identr, in_=ident)
        nc.vector.memset(vv[:, :, :, HD].bitcast(f32), 1.0)

        # ---- phase A: xT, qkT, v ----
        with (
            tc.tile_pool(name="phA", bufs=1) as phA,
            tc.tile_pool(name="xst", bufs=3) as xst,
            tc.tile_pool(name="ps_tr", bufs=2, space="PSUM") as ps_tr,
            tc.tile_pool(name="ps_qk", bufs=2, space="PSUM") as ps_qk,
            tc.tile_pool(name="ps_v", bufs=2, space="PSUM") as ps_v,
        ):
            xT = phA.tile([P, NCC, T], f32r)
            wqk_sb = phA.tile([P, NCC, 2 * CL], f32r)
            wv_sb = phA.tile([P, NCC, CL], f32r)

            # weights travel on the gpsimd (SWDGE) queue, x on the sync
            # (HWDGE) queue - they overlap instead of serializing
            nc.gpsimd.dma_start(out=wqk_sb, in_=wqk_d[:].rearrange("(o p) n -> p o n", p=P))
            nc.gpsimd.dma_start(out=wv_sb, in_=wv_d[:].rearrange("(o p) n -> p o n", p=P))
            nc.gpsimd.dma_start(out=wproj_sb, in_=wproj_d[:].rearrange("(o p) n -> p o n", p=P))

            for ti in range(NT):
                xt = xst.tile([P, C], f32r)
                nc.sync.dma_start(out=xt, in_=x_d[ti * P:(ti + 1) * P, :])
                for cb in range(NCC // 4):
                    pt = ps_tr.tile([P, 4, P], f32r)
                    for m in range(4):
                        ci = cb * 4 + m
                        nc.tensor.transpose(
                            pt[:, m, :], xt[:, ci * P:(ci + 1) * P], identr
                        )
                    if cb % 2 == 0:
                        nc.vector.tensor_copy(
                            out=xT[:, cb * 4:(cb + 1) * 4, ti * P:(ti + 1) * P],
                            in_=pt,
                        )
                    else:
                        nc.scalar.copy(
                            out=xT[:, cb * 4:(cb + 1) * 4, ti * P:(ti + 1) * P],
                            in_=pt,
                        )

            # qT,kT: stationary = W chunk, moving = xT
            for fi in range(4):
                for tj in range(4):
                    pq = ps_qk.tile([P, 512], f32)
                    for ci in range(NCC):
                        nc.tensor.matmul(
                            pq,
                            lhsT=wqk_sb[:, ci, fi * P:(fi + 1) * P],
                            rhs=xT[:, ci, tj * 512:(tj + 1) * 512],
                            start=(ci == 0),
                            stop=(ci == NCC - 1),
                        )
                    nc.vector.tensor_scalar_add(
                        out=qkT[:, fi, tj * 512:(tj + 1) * 512],
                        in0=pq,
                        scalar1=bqk_sb[:, fi:fi + 1],
                    )

            # v natural layout: stationary = xT chunk, moving = Wv
            for ti in range(NT):
                pv = ps_v.tile([P, CL], f32)
                for ci in range(NCC):
                    nc.tensor.matmul(
                        pv,
                        lhsT=xT[:, ci, ti * P:(ti + 1) * P],
                        rhs=wv_sb[:, ci, :],
                        start=(ci == 0),
                        stop=(ci == NCC - 1),
                    )
                nc.vector.tensor_add(
                    out=vv[:, ti, :, 0:HD],
                    in0=pv.rearrange("p (h d) -> p h d", h=HL),
                    in1=bv_sb,
                )
                # key-padding mask: zero this key's v row AND its ones-col
                # entry (excludes it from both numerator and denominator)
                nc.vector.tensor_scalar_mul(
                    out=vv[:, ti, :, :],
                    in0=vv[:, ti, :, :],
                    scalar1=kbias_sb[:, ti:ti + 1],
                )

        # ---- phase B+C: attention with per-quad projection ----
        with (
            tc.tile_pool(name="ptp", bufs=3) as ptp,
            tc.tile_pool(name="ep", bufs=3) as ep,
            tc.tile_pool(name="yst", bufs=3) as yst,
            tc.tile_pool(name="ps_s", bufs=2, space="PSUM") as ps_s,
            tc.tile_pool(name="ps_o", bufs=2, space="PSUM") as ps_o,
            tc.tile_pool(name="ps_y", bufs=2, space="PSUM") as ps_y,
        ):
            for qq in range(NQ):
                for h in range(HL):
                    bp = (h % 2) * HD
                    fo = h // 2
                    qTh = qkT[bp:bp + HD, fo, :]
                    kTh = qkT[bp:bp + HD, 2 + fo, :]
                    po = ps_o.tile([HD + 1, 512], f32)
                    nj = 4 * qq + 4
                    qs = qq * 512
                    # full (below-diagonal) chunk PAIRS: two S matmuls into a
                    # 2-bank psum tile, ONE exp over both (halves the per-
                    # instruction ACT overhead)
                    for jp in range(2 * qq):
                        j0 = 2 * jp
                        ps2 = ps_s.tile([P, 2, 512], f32, tag="s")
                        pT2 = ptp.tile([P, 2, 512], f32r, tag="p")
                        for m in range(2):
                            nc.tensor.matmul(
                                ps2[:, m, :],
                                lhsT=kTh[:, (j0 + m) * P:(j0 + m + 1) * P],
                                rhs=qTh[:, qs:qs + 512],
                                start=True,
                                stop=True,
                            )
                        nc.scalar.activation(
                            out=pT2, in_=ps2, func=EXP, scale=0.125,
                        )
                        for m in range(2):
                            nc.tensor.matmul(
                                po,
                                lhsT=vv[:, j0 + m, h, :],
                                rhs=pT2[:, m, :],
                                start=(j0 + m == 0),
                                stop=False,
                            )
                    # diagonal-region chunks (o = 0..3), width-trimmed
                    for o in range(4):
                        j = 4 * qq + o
                        a = min(128 * o, 256)
                        d0 = 128 * o
                        ps = ps_s.tile([P, 2, 512], f32, tag="s")
                        nc.tensor.matmul(
                            ps[:, 0, a:],
                            lhsT=kTh[:, j * P:(j + 1) * P],
                            rhs=qTh[:, qs + a:qs + 512],
                            start=True,
                            stop=True,
                        )
                        pT = ptp.tile([P, 2, 512], f32r, tag="p")
                        nc.scalar.activation(
                            out=pT[:, 0, d0:], in_=ps[:, 0, d0:],
                            func=EXP, scale=0.125,
                        )
                        nc.vector.tensor_mul(
                            out=pT[:, 0, d0:d0 + P],
                            in0=pT[:, 0, d0:d0 + P],
                            in1=tri01,
                        )
                        if d0 > a:
                            # o==3: cols [256,384) are causally invalid
                            nc.vector.memset(
                                pT[:, 0, a:d0].bitcast(f32), 0.0
                            )
                        nc.tensor.matmul(
                            po[:, a:],
                            lhsT=vv[:, j, h, :],
                            rhs=pT[:, 0, a:],
                            start=(j == 0),
                            stop=(o == 3),
                        )
                    # normalize: rows 0:64 divided by row 64 (the ones-col sum)
                    # row 64 of po = softmax denominator (ones col of vv).
                    # partition_broadcast reads PHYSICAL partition 0, so the
                    # denom row goes SBUF -> partition-0 tile via DMA first.
                    ob = ep.tile([HD + 1, 512], f32r)
                    nc.vector.tensor_copy(
                        out=ob[HD:HD + 1, :], in_=po[HD:HD + 1, :]
                    )
                    den0 = ep.tile([1, 512], f32r)
                    nc.sync.dma_start(out=den0, in_=ob[HD:HD + 1, :])
                    rb = ep.tile([HD, 512], f32r)
                    nc.gpsimd.partition_broadcast(rb, den0)
                    with nc.allow_low_precision(
                        reason="f32r reciprocal of softmax denom; 2^-11 rel"
                    ):
                        nc.vector.reciprocal(out=rb, in_=rb)
                    nc.vector.tensor_mul(
                        out=ob[0:HD, :], in0=po[0:HD, :], in1=rb
                    )
                    nc.sync.dma_start(
                        out=AT[bp:bp + HD, fo, qq * 512:(qq + 1) * 512],
                        in_=ob[0:HD, :],
                    )

                # projection for this quad's t-range; fills PE slack while
                # the next quad's attention is ACT(exp)-bound
                for co in range(C // P):
                    py = ps_y.tile([P, 512], f32)
                    for cc in range(2):
                        nc.tensor.matmul(
                            py,
                            lhsT=wproj_sb[:, cc, co * P:(co + 1) * P],
                            rhs=AT[:, cc, qq * 512:(qq + 1) * 512],
                            start=(cc == 0),
                            stop=(cc == 1),
                        )
                    yt = yst.tile([P, 512], f32)
                    if co % 2 == 0:
                        nc.scalar.copy(out=yt, in_=py)
                    else:
                        nc.vector.tensor_copy(out=yt, in_=py)
                    dma_eng = nc.sync if co % 2 == 0 else nc.gpsimd
                    dma_eng.dma_start(
                        out=yT_d[co * P:(co + 1) * P, qq * 512:(qq + 1) * 512],
                        in_=yt,
                    )

    return nc


def _get_nc():
    if "nc" not in _cache:
        nc = _build_nc()
        nc.finalize()  # runs the Bacc pass pipeline (reg alloc, wait splitting)
        _cache["nc"] = nc
    return _cache["nc"]


def _make_in_maps(x, attn_mask, W_qkv, b_qkv, W_proj):
    x = np.asarray(x, dtype=np.float32)
    attn_mask = np.asarray(attn_mask)
    W_qkv = np.asarray(W_qkv, dtype=np.float32)
    b_qkv = np.asarray(b_qkv, dtype=np.float32)
    W_proj = np.asarray(W_proj, dtype=np.float32)

    in_maps = []
    for core in range(8):
        b, g = core // 4, core % 4
        s = slice(CL * g, CL * (g + 1))
        wq = W_qkv[:, 0 * C:1 * C][:, s]
        wk = W_qkv[:, 1 * C:2 * C][:, s]
        wv = W_qkv[:, 2 * C:3 * C][:, s]
        bq = b_qkv[0 * C:1 * C][s]
        bk = b_qkv[1 * C:2 * C][s]
        bv = b_qkv[2 * C:3 * C][s]
        bqk = np.concatenate([bq, bk]).reshape(4, P).T  # [128,4], f = fi*128+p
        kbias = (attn_mask[b] != 0).astype(np.float32)  # 0/1 key mask
        in_maps.append({
            "x": np.ascontiguousarray(x[b]),
            "wqk": np.ascontiguousarray(np.concatenate([wq, wk], axis=1)),
            "wv": np.ascontiguousarray(wv),
            "bqk": np.ascontiguousarray(bqk),
            "bv": np.ascontiguousarray(bv.reshape(1, CL)),
            "kbias": np.ascontiguousarray(kbias.reshape(16, P).T),
            "wproj": np.ascontiguousarray(W_proj[s, :]),
        })
    return in_maps


def kernel(x, attn_mask, W_qkv, b_qkv, W_proj, b_proj, _trace=False):
    from concourse.bass_utils import run_bass_kernel_spmd

    nc = _get_nc()
    in_maps = _make_in_maps(x, attn_mask, W_qkv, b_qkv, W_proj)
    res = run_bass_kernel_spmd(nc, in_maps, list(range(8)), trace=_trace)
    outs = res.results

    b_proj = np.asarray(b_proj, dtype=np.float32)
    y = np.empty((2, T, C), dtype=np.float32)
    for b in range(2):
        acc = outs[b * 4]["yT"].T.astype(np.float32).copy()
        for g in range(1, 4):
            acc += outs[b * 4 + g]["yT"].T
        y[b] = acc + b_proj
    if _trace:
        return y, res
    return y



# revision 6
# speedup vs baseline: 1.1178x; 1.1178x over previous
# Causal self-attention on 8 NeuronCores (Trainium2, Bass/Tile).
#
# Problem: B=2, T=2048, C=1024, H=16 heads (hd=64).
#   qkv = x @ W_qkv + b_qkv ; per-head causal softmax attention ; y = att_out @ W_proj + b_proj
#
# Sharding: tensor-parallel over heads x data-parallel over batch.
#   core = b*4 + g   (b in {0,1} batch, g in {0..3} head group of 4 heads)
#   Each core: qkv projection for its 4 heads (W_qkv column shard),
#   attention for those heads, then a partial row-shard projection
#   y_partial^T = W_proj[g-rows]^T @ att_out^T.  Host sums the 4 partials
#   per batch and adds b_proj.
#
# On-chip layout is "transposed" (feature-on-partition) throughout so no
# P-matrix transposes are needed beyond the initial xT build:
#   xT [C, T] (PE transpose) -> qT,kT [64, T] per head, v [T, 64] per head
#   with a PREPENDED ones-column so the PV matmul also produces the softmax
#   denominator on PSUM partition 0 (where gpsimd partition_broadcast can
#   read it directly - no staging DMA), S^T [k, q] chunks (exp on ACT),
#   out^T [65, q] accumulated in PSUM rows 1:65, denom in row 0.
#
# The whole kernel is a single software-pipelined stream: the "phase A"
# work (x transposes, qk/v projections) and the output projection are
# chopped into small thunks that are interleaved between the attention
# S -> exp -> PV steps, so the PE stays busy while ACT chews on exp and
# the attention for q-quad qq starts as soon as k/v for t < (qq+1)*512
# exist.  Matmuls use float32r (full PE rate at N>=256) with fp32 acc.

import numpy as np
from collections import deque

T = 2048
C = 1024
HL = 4          # heads per core
HD = 64
CL = HL * HD    # 256 local channels
P = 128

_cache = {}


def _build_nc():
    import concourse.bass as bass
    import concourse.mybir as mybir
    import concourse.tile as tile
    from concourse import bacc
    from concourse.masks import make_identity
    from contextlib import ExitStack

    f32 = mybir.dt.float32
    f32r = mybir.dt.float32r
    EXP = mybir.ActivationFunctionType.Exp

    nc = bacc.Bacc("TRN2", target_bir_lowering=False)
    x_d = nc.declare_dram_parameter("x", [T, C], f32r, isOutput=False)
    wqk_d = nc.declare_dram_parameter("wqk", [C, 2 * CL], f32r, isOutput=False)
    wv_d = nc.declare_dram_parameter("wv", [C, CL], f32r, isOutput=False)
    bqk_d = nc.declare_dram_parameter("bqk", [P, 4], f32, isOutput=False)
    bv_d = nc.declare_dram_parameter("bv", [1, CL], f32, isOutput=False)
    kbias_d = nc.declare_dram_parameter("kbias", [P, 16], f32, isOutput=False)
    wproj_d = nc.declare_dram_parameter("wproj", [CL, C], f32r, isOutput=False)
    yT_d = nc.declare_dram_parameter("yT", [C, T], f32, isOutput=True)

    NT = T // P       # 16 t-tiles of 128
    NCC = C // P      # 8 contraction chunks of 128
    NQ = T // 512     # 4 q-quads of 512

    with tile.TileContext(nc) as tc, ExitStack() as ctx:
        singles = ctx.enter_context(tc.tile_pool(name="singles", bufs=1))

        # persistent SBUF
        qkT = singles.tile([P, 4, T], f32r)         # rows: [q f0,q f1,k f0,k f1]
        vv = singles.tile([P, NT, HL, HD + 1], f32r)  # ones col FIRST, then v
        AT = singles.tile([P, 2, T], f32r)          # attention out^T (c' x t)
        xT = singles.tile([P, NCC, T], f32r)
        wqk_sb = singles.tile([P, NCC, 2 * CL], f32r)
        wv_sb = singles.tile([P, NCC, CL], f32r)
        wproj_sb = singles.tile([P, 2, C], f32r)
        tri01 = singles.tile([P, P], f32)          # lower-tri 1.0 / 0.0
        kbias_sb = singles.tile([P, 16], f32)
        bqk_sb = singles.tile([P, 4], f32)
        bv_sb = singles.tile([P, HL, HD], f32)
        identr = singles.tile([P, P], f32r)

        make_identity(nc, identr.bitcast(f32))
        # tri01[k, q] = 1.0 where q >= k else 0.0
        nc.gpsimd.memset(tri01, 1.0)
        nc.gpsimd.affine_select(
            out=tri01,
            in_=tri01,
            compare_op=mybir.AluOpType.is_ge,
            fill=0.0,
            base=0,
            pattern=[[1, P]],
            channel_multiplier=-1,
        )

        # small/params on the gpsimd (SWDGE) queue so the big x loads on
        # the sync queue aren't delayed
        nc.gpsimd.dma_start(out=kbias_sb, in_=kbias_d[:])
        nc.gpsimd.dma_start(out=bqk_sb, in_=bqk_d[:])
        nc.gpsimd.dma_start(
            out=bv_sb,
            in_=bv_d[:].rearrange("o (h d) -> o h d", h=HL).to_broadcast([P, HL, HD]),
        )
        nc.vector.memset(vv[:, :, :, 0].bitcast(f32), 1.0)

        # weights on the gpsimd queue; wproj last (not needed until the
        # first projection ~halfway through)
        nc.gpsimd.dma_start(out=wqk_sb, in_=wqk_d[:].rearrange("(o p) n -> p o n", p=P))
        nc.gpsimd.dma_start(out=wv_sb, in_=wv_d[:].rearrange("(o p) n -> p o n", p=P))
        nc.gpsimd.dma_start(out=wproj_sb, in_=wproj_d[:].rearrange("(o p) n -> p o n", p=P))

        with (
            tc.tile_pool(name="xst", bufs=2) as xst,
            tc.tile_pool(name="pa_ps", bufs=2, space="PSUM") as pa_ps,
            tc.tile_pool(name="ps_s", bufs=2, space="PSUM") as ps_s,
            tc.tile_pool(name="ps_o", bufs=2, space="PSUM") as ps_o,
            tc.tile_pool(name="ptp", bufs=3) as ptp,
            tc.tile_pool(name="ep", bufs=2) as ep,
            tc.tile_pool(name="yst", bufs=3) as yst,
        ):
            # ---- filler thunk machinery ------------------------------------
            # Units of "phase A" / projection work are chopped into small
            # thunks tagged with the quad index that requires their data.
            # flush_stage(s) runs everything needed before quad s; emit_some
            # interleaves thunks into the attention stream as PE filler.
            FQ = deque()

            def emit_some(n):
                for _ in range(n):
                    if not FQ:
                        return
                    FQ.popleft()[1]()

            def flush_stage(s):
                while FQ and FQ[0][0] <= s:
                    FQ.popleft()[1]()

            def flush_all():
                while FQ:
                    FQ.popleft()[1]()

            def unit_T(ti, stage):
                # load x tile ti and build xT[:, :, ti*P:(ti+1)*P]
                st = {}

                def t_load():
                    st["xt"] = xst.tile([P, C], f32r, name="xt")
                    nc.sync.dma_start(out=st["xt"], in_=x_d[ti * P:(ti + 1) * P, :])

                FQ.append((stage, t_load))
                for half in (0, 1):
                    def t_tr(half=half):
                        pt = pa_ps.tile([P, 4, P], f32r, tag="pa")
                        for m in range(4):
                            ci = half * 4 + m
                            nc.tensor.transpose(
                                pt[:, m, :], st["xt"][:, ci * P:(ci + 1) * P], identr
                            )
                        eng = nc.vector if (ti + half) % 2 == 0 else nc.gpsimd
                        eng.tensor_copy(
                            out=xT[:, half * 4:(half + 1) * 4, ti * P:(ti + 1) * P],
                            in_=pt,
                        )

                    FQ.append((stage, t_tr))

            def unit_QK(fi, tj, stage):
                # qkT[:, fi, tj*512:(tj+1)*512] = W chunk^T @ xT + bias
                st = {}

                def q_first():
                    st["pq"] = pa_ps.tile([P, 512], f32, tag="pa", name="pq")
                    for ci in range(2):
                        nc.tensor.matmul(
                            st["pq"],
                            lhsT=wqk_sb[:, ci, fi * P:(fi + 1) * P],
                            rhs=xT[:, ci, tj * 512:(tj + 1) * 512],
                            start=(ci == 0),
                            stop=False,
                        )

                FQ.append((stage, q_first))
                for cb in (2, 4, 6):
                    def q_mid(cb=cb):
                        for ci in range(cb, cb + 2):
                            nc.tensor.matmul(
                                st["pq"],
                                lhsT=wqk_sb[:, ci, fi * P:(fi + 1) * P],
                                rhs=xT[:, ci, tj * 512:(tj + 1) * 512],
                                start=False,
                                stop=(ci == NCC - 1),
                            )

                    FQ.append((stage, q_mid))

                def q_bias():
                    nc.vector.tensor_scalar_add(
                        out=qkT[:, fi, tj * 512:(tj + 1) * 512],
                        in0=st["pq"],
                        scalar1=bqk_sb[:, fi:fi + 1],
                    )

                FQ.append((stage, q_bias))

            def unit_V(tp, stage):
                # v for t-tiles 2tp, 2tp+1 -> vv[:, ti, :, 1:65]
                st = {}

                def v_first():
                    st["pv"] = pa_ps.tile([P, 2, CL], f32, tag="pa", name="pv")
                    for ci in range(4):
                        nc.tensor.matmul(
                            st["pv"][:, 0, :],
                            lhsT=xT[:, ci, 2 * tp * P:(2 * tp + 1) * P],
                            rhs=wv_sb[:, ci, :],
                            start=(ci == 0),
                            stop=False,
                        )

                FQ.append((stage, v_first))

                def v_second():
                    for ci in range(4, NCC):
                        nc.tensor.matmul(
                            st["pv"][:, 0, :],
                            lhsT=xT[:, ci, 2 * tp * P:(2 * tp + 1) * P],
                            rhs=wv_sb[:, ci, :],
                            start=False,
                            stop=(ci == NCC - 1),
                        )

                FQ.append((stage, v_second))

                def v_third():
                    for ci in range(4):
                        nc.tensor.matmul(
                            st["pv"][:, 1, :],
                            lhsT=xT[:, ci, (2 * tp + 1) * P:(2 * tp + 2) * P],
                            rhs=wv_sb[:, ci, :],
                            start=(ci == 0),
                            stop=False,
                        )

                FQ.append((stage, v_third))

                def v_fourth():
                    for ci in range(4, NCC):
                        nc.tensor.matmul(
                            st["pv"][:, 1, :],
                            lhsT=xT[:, ci, (2 * tp + 1) * P:(2 * tp + 2) * P],
                            rhs=wv_sb[:, ci, :],
                            start=False,
                            stop=(ci == NCC - 1),
                        )

                FQ.append((stage, v_fourth))

                def v_bias(k, ti):
                    nc.vector.tensor_add(
                        out=vv[:, ti, :, 1:HD + 1],
                        in0=st["pv"][:, k, :].rearrange("p (h d) -> p h d", h=HL),
                        in1=bv_sb,
                    )
                    # key-padding mask: zero this key's v row AND its ones-col
                    # entry (excludes it from numerator and denominator)
                    nc.vector.tensor_scalar_mul(
                        out=vv[:, ti, :, :],
                        in0=vv[:, ti, :, :],
                        scalar1=kbias_sb[:, ti:ti + 1],
                    )

                FQ.append((stage, lambda: v_bias(0, 2 * tp)))
                FQ.append((stage, lambda: v_bias(1, 2 * tp + 1)))

            def unit_PR(qq, stage):
                # projection for quad qq: yT[:, qq*512:(qq+1)*512]
                for co in range(C // P):
                    def pr(co=co):
                        py = pa_ps.tile([P, 512], f32, tag="pa")
                        for cc in range(2):
                            nc.tensor.matmul(
                                py,
                                lhsT=wproj_sb[:, cc, co * P:(co + 1) * P],
                                rhs=AT[:, cc, qq * 512:(qq + 1) * 512],
                                start=(cc == 0),
                                stop=(cc == 1),
                            )
                        yt = yst.tile([P, 512], f32)
                        ceng = nc.vector if co % 2 == 0 else nc.gpsimd
                        ceng.tensor_copy(out=yt, in_=py)
                        deng = nc.sync if co % 2 == 0 else nc.gpsimd
                        deng.dma_start(
                            out=yT_d[co * P:(co + 1) * P, qq * 512:(qq + 1) * 512],
                            in_=yt,
                        )

                    FQ.append((stage, pr))

            # ---- attention for one (quad, head) ----------------------------
            def attn(qq, h):
                bp = (h % 2) * HD
                fo = h // 2
                qTh = qkT[bp:bp + HD, fo, :]
                kTh = qkT[bp:bp + HD, 2 + fo, :]
                po = ps_o.tile([HD + 1, 512], f32)
                qs = qq * 512
                # full (below-diagonal) chunk PAIRS
                for jp in range(2 * qq):
                    j0 = 2 * jp
                    ps2 = ps_s.tile([P, 2, 512], f32, tag="s")
                    for m in range(2):
                        nc.tensor.matmul(
                            ps2[:, m, :],
                            lhsT=kTh[:, (j0 + m) * P:(j0 + m + 1) * P],
                            rhs=qTh[:, qs:qs + 512],
                            start=True,
                            stop=True,
                        )
                    pT2 = ptp.tile([P, 2, 512], f32r, tag="p")
                    nc.scalar.activation(out=pT2, in_=ps2, func=EXP, scale=0.125)
                    emit_some(1)
                    for m in range(2):
                        nc.tensor.matmul(
                            po,
                            lhsT=vv[:, j0 + m, h, :],
                            rhs=pT2[:, m, :],
                            start=(j0 + m == 0),
                            stop=False,
                        )
                    emit_some(1)
                # diagonal-region chunks o=0..3 (keys jb..jb+3), packed into
                # two ps_s tiles so the exp batches:
                #   tile A: o0 @ [0:512] (q 0:512), o1 @ [512:896] (q 128:512)
                #   tile B: o2 @ [0:256] (q 256:512), o3 @ [256:512] (q 256:512)
                jb = 4 * qq
                A = ps_s.tile([P, 2, 512], f32, tag="s")
                Af = A.rearrange("p a b -> p (a b)")
                nc.tensor.matmul(
                    A[:, 0, :], lhsT=kTh[:, jb * P:(jb + 1) * P],
                    rhs=qTh[:, qs:qs + 512], start=True, stop=True,
                )
                nc.tensor.matmul(
                    Af[:, 512:896], lhsT=kTh[:, (jb + 1) * P:(jb + 2) * P],
                    rhs=qTh[:, qs + 128:qs + 512], start=True, stop=True,
                )
                pTA = ptp.tile([P, 2, 512], f32r, tag="p")
                pTAf = pTA.rearrange("p a b -> p (a b)")
                nc.scalar.activation(
                    out=pTAf[:, 0:896], in_=Af[:, 0:896], func=EXP, scale=0.125,
                )
                emit_some(1)
                # causal tri-mask on the first 128 cols of each diag block
                nc.vector.tensor_mul(
                    out=pTA[:, 0, 0:P], in0=pTA[:, 0, 0:P], in1=tri01,
                )
                nc.gpsimd.tensor_mul(
                    out=pTAf[:, 512:512 + P], in0=pTAf[:, 512:512 + P], in1=tri01,
                )
                nc.tensor.matmul(
                    po, lhsT=vv[:, jb, h, :], rhs=pTA[:, 0, :],
                    start=(jb == 0), stop=False,
                )
                nc.tensor.matmul(
                    po[:, 128:], lhsT=vv[:, jb + 1, h, :], rhs=pTAf[:, 512:896],
                    start=False, stop=False,
                )
                B = ps_s.tile([P, 2, 512], f32, tag="s")
                nc.tensor.matmul(
                    B[:, 0, 0:256], lhsT=kTh[:, (jb + 2) * P:(jb + 3) * P],
                    rhs=qTh[:, qs + 256:qs + 512], start=True, stop=True,
                )
                nc.tensor.matmul(
                    B[:, 0, 256:512], lhsT=kTh[:, (jb + 3) * P:(jb + 4) * P],
                    rhs=qTh[:, qs + 256:qs + 512], start=True, stop=True,
                )
                pTB = ptp.tile([P, 2, 512], f32r, tag="p")
                nc.scalar.activation(
                    out=pTB[:, 0, :], in_=B[:, 0, :], func=EXP, scale=0.125,
                )
                emit_some(1)
                # o3 cols [256:384] (q 256:384 vs keys >= 384) are causally
                # invalid; zero them so the padded-width PV adds nothing
                nc.vector.memset(pTB[:, 0, 256:384].bitcast(f32), 0.0)
                nc.vector.tensor_mul(
                    out=pTB[:, 0, 0:P], in0=pTB[:, 0, 0:P], in1=tri01,
                )
                nc.gpsimd.tensor_mul(
                    out=pTB[:, 0, 384:512], in0=pTB[:, 0, 384:512], in1=tri01,
                )
                nc.tensor.matmul(
                    po[:, 256:], lhsT=vv[:, jb + 2, h, :], rhs=pTB[:, 0, 0:256],
                    start=False, stop=False,
                )
                nc.tensor.matmul(
                    po[:, 256:], lhsT=vv[:, jb + 3, h, :], rhs=pTB[:, 0, 256:512],
                    start=False, stop=True,
                )
                # normalize: rows 1:65 divided by row 0 (the ones-col sum).
                # The denom lives on PSUM partition 0, so partition_broadcast
                # can fan out its reciprocal without any staging DMA.  The
                # out rows sit at unaligned partitions 1:65, so they are
                # DMA'd (no partition rules) unnormalized into AT, and the
                # normalize mul runs in place on AT's aligned rows.
                rcp = ep.tile([1, 512], f32r, tag="rcp")
                with nc.allow_low_precision(
                    reason="f32r reciprocal of softmax denom; 2^-11 rel"
                ):
                    nc.vector.reciprocal(out=rcp, in_=po[0:1, :])
                ob = ep.tile([HD + 1, 512], f32r, tag="ob")
                ceng = nc.vector if h % 2 == 0 else nc.gpsimd
                ceng.tensor_copy(out=ob, in_=po)
                deng = nc.sync if h % 2 == 0 else nc.gpsimd
                deng.dma_start(
                    out=AT[bp:bp + HD, fo, qs:qs + 512], in_=ob[1:HD + 1, :],
                )
                rb = ep.tile([P, 512], f32r, tag="rb")
                nc.gpsimd.partition_broadcast(rb, rcp)
                nc.vector.tensor_mul(
                    out=AT[bp:bp + HD, fo, qs:qs + 512],
                    in0=AT[bp:bp + HD, fo, qs:qs + 512],
                    in1=rb[bp:bp + HD, :],
                )
                emit_some(2)

            # ---- schedule --------------------------------------------------
            # stage-0 prologue (everything quad 0 needs)
            for ti in range(4):
                unit_T(ti, 0)
            unit_QK(0, 0, 0)
            unit_QK(2, 0, 0)
            unit_V(0, 0)
            unit_V(1, 0)
            unit_QK(1, 0, 0)
            unit_QK(3, 0, 0)
            flush_stage(0)

            for qq in range(NQ):
                if qq < NQ - 1:
                    s = qq + 1
                    for ti in range(4 * s, 4 * s + 4):
                        unit_T(ti, s)
                    unit_QK(0, s, s)
                    unit_QK(2, s, s)
                    unit_V(2 * s, s)
                    unit_QK(1, s, s)
                    unit_V(2 * s + 1, s)
                    unit_QK(3, s, s)
                if qq >= 1:
                    # projection of the previous quad interleaves as filler
                    # into this quad and must flush before quad qq+1 ends
                    unit_PR(qq - 1, qq + 2)
                for h in range(HL):
                    attn(qq, h)
                if qq < NQ - 1:
                    flush_stage(qq + 1)
            flush_all()
            unit_PR(NQ - 1, 99)
            flush_all()

    return nc


def _get_nc():
    if "nc" not in _cache:
        nc = _build_nc()
        nc.finalize()  # runs the Bacc pass pipeline (reg alloc, wait splitting)
        _cache["nc"] = nc
    return _cache["nc"]


def _make_in_maps(x, attn_mask, W_qkv, b_qkv, W_proj):
    x = np.asarray(x, dtype=np.float32)
    attn_mask = np.asarray(attn_mask)
    W_qkv = np.asarray(W_qkv, dtype=np.float32)
    b_qkv = np.asarray(b_qkv, dtype=np.float32)
    W_proj = np.asarray(W_proj, dtype=np.float32)

    in_maps = []
    for core in range(8):
        b, g = core // 4, core % 4
        s = slice(CL * g, CL * (g + 1))
        wq = W_qkv[:, 0 * C:1 * C][:, s]
        wk = W_qkv[:, 1 * C:2 * C][:, s]
        wv = W_qkv[:, 2 * C:3 * C][:, s]
        bq = b_qkv[0 * C:1 * C][s]
        bk = b_qkv[1 * C:2 * C][s]
        bv = b_qkv[2 * C:3 * C][s]
        bqk = np.concatenate([bq, bk]).reshape(4, P).T  # [128,4], f = fi*128+p
        kbias = (attn_mask[b] != 0).astype(np.float32)  # 0/1 key mask
        in_maps.append({
            "x": np.ascontiguousarray(x[b]),
            "wqk": np.ascontiguousarray(np.concatenate([wq, wk], axis=1)),
            "wv": np.ascontiguousarray(wv),
            "bqk": np.ascontiguousarray(bqk),
            "bv": np.ascontiguousarray(bv.reshape(1, CL)),
            "kbias": np.ascontiguousarray(kbias.reshape(16, P).T),
            "wproj": np.ascontiguousarray(W_proj[s, :]),
        })
    return in_maps


def kernel(x, attn_mask, W_qkv, b_qkv, W_proj, b_proj, _trace=False):
    from concourse.bass_utils import run_bass_kernel_spmd

    nc = _get_nc()
    in_maps = _make_in_maps(x, attn_mask, W_qkv, b_qkv, W_proj)
    res = run_bass_kernel_spmd(nc, in_maps, list(range(8)), trace=_trace)
    outs = res.results

    b_proj = np.asarray(b_proj, dtype=np.float32)
    y = np.empty((2, T, C), dtype=np.float32)
    for b in range(2):
        acc = outs[b * 4]["yT"].T.astype(np.float32).copy()
        for g in range(1, 4):
            acc += outs[b * 4 + g]["yT"].T
        y[b] = acc + b_proj
    if _trace:
        return y, res
    return y


# revision 11
# speedup vs baseline: 1.1601x; 1.0379x over previous
# Causal self-attention on 8 NeuronCores (Trainium2, Bass/Tile).
#
# Problem: B=2, T=2048, C=1024, H=16 heads (hd=64).
#   qkv = x @ W_qkv + b_qkv ; per-head causal softmax attention ; y = att_out @ W_proj + b_proj
#
# Sharding: tensor-parallel over heads x data-parallel over batch.
#   core = b*4 + g   (b in {0,1} batch, g in {0..3} head group of 4 heads)
#   Each core: qkv projection for its 4 heads (W_qkv column shard),
#   attention for those heads, then a partial row-shard projection
#   y_partial^T = W_proj[g-rows]^T @ att_out^T.  Host sums the 4 partials
#   per batch and adds b_proj.
#
# On-chip layout is "transposed" (feature-on-partition) throughout so no
# P-matrix transposes are needed beyond the initial xT build:
#   xT [C, T] (PE transpose) -> qT,kT [64, T] per head, v [T, 64] per head
#   with a PREPENDED ones-column so the PV matmul also produces the softmax
#   denominator on PSUM partition 0 (where gpsimd partition_broadcast can
#   read it directly - no staging DMA), S^T [k, q] chunks (exp on ACT),
#   out^T [65, q] accumulated in PSUM rows 1:65, denom in row 0.
#
# The whole kernel is a single software-pipelined stream: the "phase A"
# work (x transposes, qk/v projections) and the output projection are
# chopped into small thunks that are interleaved between the attention
# S -> exp -> PV steps, so the PE stays busy while ACT chews on exp and
# the attention for q-quad qq starts as soon as k/v for t < (qq+1)*512
# exist.  Matmuls use float32r (full PE rate at N>=256) with fp32 acc.

import numpy as np
from collections import deque

T = 2048
C = 1024
HL = 4          # heads per core
HD = 64
CL = HL * HD    # 256 local channels
P = 128

_cache = {}


def _build_nc():
    import concourse.bass as bass
    import concourse.mybir as mybir
    import concourse.tile as tile
    from concourse import bacc
    from concourse.masks import make_identity
    from contextlib import ExitStack

    f32 = mybir.dt.float32
    f32r = mybir.dt.float32r
    EXP = mybir.ActivationFunctionType.Exp

    nc = bacc.Bacc("TRN2", target_bir_lowering=False)
    x_d = nc.declare_dram_parameter("x", [T, C], f32r, isOutput=False)
    wqk_d = nc.declare_dram_parameter("wqk", [C, 2 * CL], f32r, isOutput=False)
    wv_d = nc.declare_dram_parameter("wv", [C, CL], f32r, isOutput=False)
    bqk_d = nc.declare_dram_parameter("bqk", [P, 4], f32, isOutput=False)
    bv_d = nc.declare_dram_parameter("bv", [1, CL], f32, isOutput=False)
    kbias_d = nc.declare_dram_parameter("kbias", [P, 16], f32, isOutput=False)
    wproj_d = nc.declare_dram_parameter("wproj", [CL, C], f32r, isOutput=False)
    yT_d = nc.declare_dram_parameter("yT", [C, T], f32, isOutput=True)

    NT = T // P       # 16 t-tiles of 128
    NCC = C // P      # 8 contraction chunks of 128
    NQ = T // 512     # 4 q-quads of 512

    with tile.TileContext(nc) as tc, ExitStack() as ctx:
        singles = ctx.enter_context(tc.tile_pool(name="singles", bufs=1))

        # persistent SBUF
        qkT = singles.tile([P, 4, T], f32r)         # rows: [q f0,q f1,k f0,k f1]
        vv = singles.tile([P, NT, HL, HD + 1], f32r)  # ones col FIRST, then v
        AT = singles.tile([P, 2, T], f32r)          # attention out^T (c' x t)
        xT = singles.tile([P, NCC, T], f32r)
        wqk_sb = singles.tile([P, NCC, 2 * CL], f32r)
        wv_sb = singles.tile([P, NCC, CL], f32r)
        wproj_sb = singles.tile([P, 2, C], f32r)
        tri01 = singles.tile([P, P], f32)          # lower-tri 1.0 / 0.0
        kbias_sb = singles.tile([P, 16], f32)
        bqk_sb = singles.tile([P, 4], f32)
        bv_sb = singles.tile([P, HL, HD], f32)
        identr = singles.tile([P, P], f32r)

        make_identity(nc, identr.bitcast(f32))
        # tri01[k, q] = 1.0 where q >= k else 0.0
        nc.gpsimd.memset(tri01, 1.0)
        nc.gpsimd.affine_select(
            out=tri01,
            in_=tri01,
            compare_op=mybir.AluOpType.is_ge,
            fill=0.0,
            base=0,
            pattern=[[1, P]],
            channel_multiplier=-1,
        )

        # small/params on the gpsimd (SWDGE) queue so the big x loads on
        # the sync queue aren't delayed
        nc.gpsimd.dma_start(out=kbias_sb, in_=kbias_d[:])
        nc.gpsimd.dma_start(out=bqk_sb, in_=bqk_d[:])
        nc.gpsimd.dma_start(
            out=bv_sb,
            in_=bv_d[:].rearrange("o (h d) -> o h d", h=HL).to_broadcast([P, HL, HD]),
        )
        nc.vector.memset(vv[:, :, :, 0].bitcast(f32), 1.0)

        # weights load in chunks as filler thunks (unit_W* below) so the
        # gpsimd queue doesn't serialize 12.6us of weight DMA at startup

        with (
            tc.tile_pool(name="xst", bufs=2) as xst,
            tc.tile_pool(name="pa_ps", bufs=2, space="PSUM") as pa_ps,
            tc.tile_pool(name="ps_s", bufs=2, space="PSUM") as ps_s,
            tc.tile_pool(name="ps_o", bufs=2, space="PSUM") as ps_o,
            tc.tile_pool(name="ptp", bufs=3) as ptp,
            tc.tile_pool(name="ep", bufs=2) as ep,
            tc.tile_pool(name="yst", bufs=3) as yst,
        ):
            # ---- filler thunk machinery ------------------------------------
            # Units of "phase A" / projection work are chopped into small
            # thunks tagged with the quad index that requires their data.
            # flush_stage(s) runs everything needed before quad s; emit_some
            # interleaves thunks into the attention stream as PE filler.
            FQ = deque()

            def emit_some(n):
                for _ in range(n):
                    if not FQ:
                        return
                    FQ.popleft()[1]()

            def flush_stage(s):
                while FQ and FQ[0][0] <= s:
                    FQ.popleft()[1]()

            def flush_all():
                while FQ:
                    FQ.popleft()[1]()

            def unit_Wqk(fi, stage):
                def w():
                    nc.gpsimd.dma_start(
                        out=wqk_sb[:, :, fi * P:(fi + 1) * P],
                        in_=wqk_d[:, fi * P:(fi + 1) * P].rearrange(
                            "(o p) n -> p o n", p=P),
                    )

                FQ.append((stage, w))

            def unit_Wv(half, stage):
                def w():
                    nc.gpsimd.dma_start(
                        out=wv_sb[:, half * 4:(half + 1) * 4, :],
                        in_=wv_d[half * 512:(half + 1) * 512, :].rearrange(
                            "(o p) n -> p o n", p=P),
                    )

                FQ.append((stage, w))

            def unit_Wproj(cc, stage):
                def w():
                    nc.gpsimd.dma_start(
                        out=wproj_sb[:, cc, :],
                        in_=wproj_d[cc * P:(cc + 1) * P, :].rearrange(
                            "(o p) n -> p o n", p=P),
                    )

                FQ.append((stage, w))

            def unit_T(ti, stage):
                # load x tile ti and build xT[:, :, ti*P:(ti+1)*P]
                st = {}

                def t_load():
                    st["xt"] = xst.tile([P, C], f32r, name="xt")
                    # early tiles alternate between the sync and scalar
                    # HWDGE queues so the serial load latency halves
                    deng = nc.scalar if (ti < 8 and ti % 2 == 0) else nc.sync
                    deng.dma_start(out=st["xt"], in_=x_d[ti * P:(ti + 1) * P, :])

                FQ.append((stage, t_load))
                for half in (0, 1):
                    def t_tr(half=half):
                        pt = pa_ps.tile([P, 4, P], f32r, tag="pa")
                        for m in range(4):
                            ci = half * 4 + m
                            nc.tensor.transpose(
                                pt[:, m, :], st["xt"][:, ci * P:(ci + 1) * P], identr
                            )
                        eng = nc.vector if (ti + half) % 2 == 0 else nc.gpsimd
                        eng.tensor_copy(
                            out=xT[:, half * 4:(half + 1) * 4, ti * P:(ti + 1) * P],
                            in_=pt,
                        )

                    FQ.append((stage, t_tr))

            def unit_QK(fi, tj, stage):
                # qkT[:, fi, tj*512:(tj+1)*512] = W chunk^T @ xT + bias
                st = {}

                def q_first():
                    st["pq"] = pa_ps.tile([P, 512], f32, tag="pa", name="pq")
                    for ci in range(2):
                        nc.tensor.matmul(
                            st["pq"],
                            lhsT=wqk_sb[:, ci, fi * P:(fi + 1) * P],
                            rhs=xT[:, ci, tj * 512:(tj + 1) * 512],
                            start=(ci == 0),
                            stop=False,
                        )

                FQ.append((stage, q_first))
                for cb in (2, 4, 6):
                    def q_mid(cb=cb):
                        for ci in range(cb, cb + 2):
                            nc.tensor.matmul(
                                st["pq"],
                                lhsT=wqk_sb[:, ci, fi * P:(fi + 1) * P],
                                rhs=xT[:, ci, tj * 512:(tj + 1) * 512],
                                start=False,
                                stop=(ci == NCC - 1),
                            )

                    FQ.append((stage, q_mid))

                def q_bias():
                    eng = nc.vector if fi % 2 == 0 else nc.gpsimd
                    eng.tensor_scalar_add(
                        out=qkT[:, fi, tj * 512:(tj + 1) * 512],
                        in0=st["pq"],
                        scalar1=bqk_sb[:, fi:fi + 1],
                    )

                FQ.append((stage, q_bias))

            def unit_V(tp, stage):
                # v for t-tiles 2tp, 2tp+1 -> vv[:, ti, :, 1:65]
                st = {}

                def v_first():
                    st["pv"] = pa_ps.tile([P, 2, CL], f32, tag="pa", name="pv")
                    for ci in range(4):
                        nc.tensor.matmul(
                            st["pv"][:, 0, :],
                            lhsT=xT[:, ci, 2 * tp * P:(2 * tp + 1) * P],
                            rhs=wv_sb[:, ci, :],
                            start=(ci == 0),
                            stop=False,
                        )

                FQ.append((stage, v_first))

                def v_second():
                    for ci in range(4, NCC):
                        nc.tensor.matmul(
                            st["pv"][:, 0, :],
                            lhsT=xT[:, ci, 2 * tp * P:(2 * tp + 1) * P],
                            rhs=wv_sb[:, ci, :],
                            start=False,
                            stop=(ci == NCC - 1),
                        )

                FQ.append((stage, v_second))

                def v_third():
                    for ci in range(4):
                        nc.tensor.matmul(
                            st["pv"][:, 1, :],
                            lhsT=xT[:, ci, (2 * tp + 1) * P:(2 * tp + 2) * P],
                            rhs=wv_sb[:, ci, :],
                            start=(ci == 0),
                            stop=False,
                        )

                FQ.append((stage, v_third))

                def v_fourth():
                    for ci in range(4, NCC):
                        nc.tensor.matmul(
                            st["pv"][:, 1, :],
                            lhsT=xT[:, ci, (2 * tp + 1) * P:(2 * tp + 2) * P],
                            rhs=wv_sb[:, ci, :],
                            start=False,
                            stop=(ci == NCC - 1),
                        )

                FQ.append((stage, v_fourth))

                def v_bias(k, ti):
                    eng = nc.vector if k == 0 else nc.gpsimd
                    eng.tensor_add(
                        out=vv[:, ti, :, 1:HD + 1],
                        in0=st["pv"][:, k, :].rearrange("p (h d) -> p h d", h=HL),
                        in1=bv_sb,
                    )
                    # key-padding mask: zero this key's v row AND its ones-col
                    # entry (excludes it from numerator and denominator)
                    eng.tensor_scalar_mul(
                        out=vv[:, ti, :, :],
                        in0=vv[:, ti, :, :],
                        scalar1=kbias_sb[:, ti:ti + 1],
                    )

                FQ.append((stage, lambda: v_bias(0, 2 * tp)))
                FQ.append((stage, lambda: v_bias(1, 2 * tp + 1)))

            def unit_PR(qq, stage):
                # projection for quad qq: yT[:, qq*512:(qq+1)*512]
                for co in range(C // P):
                    def pr(co=co):
                        py = pa_ps.tile([P, 512], f32, tag="pa")
                        for cc in range(2):
                            nc.tensor.matmul(
                                py,
                                lhsT=wproj_sb[:, cc, co * P:(co + 1) * P],
                                rhs=AT[:, cc, qq * 512:(qq + 1) * 512],
                                start=(cc == 0),
                                stop=(cc == 1),
                            )
                        yt = yst.tile([P, 512], f32)
                        ceng = nc.vector if co % 2 == 0 else nc.gpsimd
                        ceng.tensor_copy(out=yt, in_=py)
                        deng = nc.sync if co % 2 == 0 else nc.gpsimd
                        deng.dma_start(
                            out=yT_d[co * P:(co + 1) * P, qq * 512:(qq + 1) * 512],
                            in_=yt,
                        )

                    FQ.append((stage, pr))

            # ---- attention for one (quad, head) ----------------------------
            def attn(qq, h):
                bp = (h % 2) * HD
                fo = h // 2
                qTh = qkT[bp:bp + HD, fo, :]
                kTh = qkT[bp:bp + HD, 2 + fo, :]
                po = ps_o.tile([HD + 1, 512], f32)
                qs = qq * 512
                # full (below-diagonal) chunk PAIRS
                for jp in range(2 * qq):
                    j0 = 2 * jp
                    ps2 = ps_s.tile([P, 2, 512], f32, tag="s")
                    for m in range(2):
                        nc.tensor.matmul(
                            ps2[:, m, :],
                            lhsT=kTh[:, (j0 + m) * P:(j0 + m + 1) * P],
                            rhs=qTh[:, qs:qs + 512],
                            start=True,
                            stop=True,
                        )
                    pT2 = ptp.tile([P, 2, 512], f32r, tag="p")
                    nc.scalar.activation(out=pT2, in_=ps2, func=EXP, scale=0.125)
                    emit_some(1)
                    for m in range(2):
                        nc.tensor.matmul(
                            po,
                            lhsT=vv[:, j0 + m, h, :],
                            rhs=pT2[:, m, :],
                            start=(j0 + m == 0),
                            stop=False,
                        )
                    emit_some(1)
                # diagonal-region chunks o=0..3 (keys jb..jb+3), packed into
                # two ps_s tiles so the exp batches:
                #   tile A: o0 @ [0:512] (q 0:512), o1 @ [512:896] (q 128:512)
                #   tile B: o2 @ [0:256] (q 256:512), o3 @ [256:512] (q 256:512)
                jb = 4 * qq
                A = ps_s.tile([P, 2, 512], f32, tag="s")
                Af = A.rearrange("p a b -> p (a b)")
                nc.tensor.matmul(
                    A[:, 0, :], lhsT=kTh[:, jb * P:(jb + 1) * P],
                    rhs=qTh[:, qs:qs + 512], start=True, stop=True,
                )
                nc.tensor.matmul(
                    Af[:, 512:896], lhsT=kTh[:, (jb + 1) * P:(jb + 2) * P],
                    rhs=qTh[:, qs + 128:qs + 512], start=True, stop=True,
                )
                pTA = ptp.tile([P, 2, 512], f32r, tag="p")
                pTAf = pTA.rearrange("p a b -> p (a b)")
                nc.scalar.activation(
                    out=pTAf[:, 0:896], in_=Af[:, 0:896], func=EXP, scale=0.125,
                )
                emit_some(1)
                # causal tri-mask on the first 128 cols of each diag block
                nc.vector.tensor_mul(
                    out=pTA[:, 0, 0:P], in0=pTA[:, 0, 0:P], in1=tri01,
                )
                nc.gpsimd.tensor_mul(
                    out=pTAf[:, 512:512 + P], in0=pTAf[:, 512:512 + P], in1=tri01,
                )
                nc.tensor.matmul(
                    po, lhsT=vv[:, jb, h, :], rhs=pTA[:, 0, :],
                    start=(jb == 0), stop=False,
                )
                nc.tensor.matmul(
                    po[:, 128:], lhsT=vv[:, jb + 1, h, :], rhs=pTAf[:, 512:896],
                    start=False, stop=False,
                )
                B = ps_s.tile([P, 2, 512], f32, tag="s")
                nc.tensor.matmul(
                    B[:, 0, 0:256], lhsT=kTh[:, (jb + 2) * P:(jb + 3) * P],
                    rhs=qTh[:, qs + 256:qs + 512], start=True, stop=True,
                )
                nc.tensor.matmul(
                    B[:, 0, 256:512], lhsT=kTh[:, (jb + 3) * P:(jb + 4) * P],
                    rhs=qTh[:, qs + 256:qs + 512], start=True, stop=True,
                )
                pTB = ptp.tile([P, 2, 512], f32r, tag="p")
                nc.scalar.activation(
                    out=pTB[:, 0, :], in_=B[:, 0, :], func=EXP, scale=0.125,
                )
                emit_some(1)
                # o3 cols [256:384] (q 256:384 vs keys >= 384) are causally
                # invalid; zero them so the padded-width PV adds nothing
                nc.vector.memset(pTB[:, 0, 256:384].bitcast(f32), 0.0)
                nc.vector.tensor_mul(
                    out=pTB[:, 0, 0:P], in0=pTB[:, 0, 0:P], in1=tri01,
                )
                nc.gpsimd.tensor_mul(
                    out=pTB[:, 0, 384:512], in0=pTB[:, 0, 384:512], in1=tri01,
                )
                nc.tensor.matmul(
                    po[:, 256:], lhsT=vv[:, jb + 2, h, :], rhs=pTB[:, 0, 0:256],
                    start=False, stop=False,
                )
                nc.tensor.matmul(
                    po[:, 256:], lhsT=vv[:, jb + 3, h, :], rhs=pTB[:, 0, 256:512],
                    start=False, stop=True,
                )
                # normalize: rows 1:65 divided by row 0 (the ones-col sum).
                # The denom lives on PSUM partition 0, so partition_broadcast
                # can fan out its reciprocal without any staging DMA.  The
                # out rows sit at unaligned partitions 1:65, so they are
                # DMA'd (no partition rules) unnormalized into AT, and the
                # normalize mul runs in place on AT's aligned rows.
                rcp = ep.tile([1, 512], f32r, tag="rcp")
                with nc.allow_low_precision(
                    reason="f32r reciprocal of softmax denom; 2^-11 rel"
                ):
                    nc.vector.reciprocal(out=rcp, in_=po[0:1, :])
                ob = ep.tile([HD + 1, 512], f32r, tag="ob")
                ceng = nc.vector if h % 2 == 0 else nc.gpsimd
                ceng.tensor_copy(out=ob, in_=po)
                deng = nc.sync if h % 2 == 0 else nc.gpsimd
                deng.dma_start(
                    out=AT[bp:bp + HD, fo, qs:qs + 512], in_=ob[1:HD + 1, :],
                )
                rb = ep.tile([P, 512], f32r, tag="rb")
                nc.gpsimd.partition_broadcast(rb, rcp)
                nc.vector.tensor_mul(
                    out=AT[bp:bp + HD, fo, qs:qs + 512],
                    in0=AT[bp:bp + HD, fo, qs:qs + 512],
                    in1=rb[bp:bp + HD, :],
                )
                emit_some(2)

            # ---- schedule --------------------------------------------------
            # preload the ACT exp table (~1.3us) while the PE warms up
            warm = ep.tile([1, 8], f32r, tag="rcp", name="warm")
            nc.scalar.activation(out=warm, in_=tri01[0:1, 0:8], func=EXP)

            # stage-0 prologue (everything quad 0 needs); weight chunks
            # interleave with the x loads/transposes
            unit_Wqk(0, 0)
            unit_T(0, 0)
            unit_Wqk(2, 0)
            unit_T(1, 0)
            unit_Wv(0, 0)
            unit_T(2, 0)
            unit_Wv(1, 0)
            unit_T(3, 0)
            unit_QK(0, 0, 0)
            unit_QK(2, 0, 0)
            unit_V(0, 0)
            unit_Wqk(1, 0)
            unit_V(1, 0)
            unit_Wqk(3, 0)
            unit_QK(1, 0, 0)
            unit_QK(3, 0, 0)
            flush_stage(0)

            for qq in range(NQ):
                if qq < NQ - 1:
                    s = qq + 1
                    for ti in range(4 * s, 4 * s + 4):
                        unit_T(ti, s)
                    if s == 1:
                        unit_Wproj(0, s)
                    unit_QK(0, s, s)
                    unit_QK(2, s, s)
                    if s == 1:
                        unit_Wproj(1, s)
                    unit_V(2 * s, s)
                    unit_QK(1, s, s)
                    unit_V(2 * s + 1, s)
                    unit_QK(3, s, s)
                if 1 <= qq <= 2:
                    # projection of the previous quad interleaves as filler
                    # into this quad and must flush before quad qq+1 ends
                    unit_PR(qq - 1, qq + 2)
                for h in range(HL):
                    attn(qq, h)
                if qq == NQ - 1:
                    # PR(2) emits after the last quad's heads: its matmuls
                    # keep the PE busy while the final head's normalize
                    # chain completes, so PR(3) then flows without a stall
                    unit_PR(qq - 1, 99)
                if qq < NQ - 1:
                    flush_stage(qq + 1)
            flush_all()
            unit_PR(NQ - 1, 99)
            flush_all()

    return nc


def _get_nc():
    if "nc" not in _cache:
        nc = _build_nc()
        nc.finalize()  # runs the Bacc pass pipeline (reg alloc, wait splitting)
        _cache["nc"] = nc
    return _cache["nc"]


def _make_in_maps(x, attn_mask, W_qkv, b_qkv, W_proj):
    x = np.asarray(x, dtype=np.float32)
    attn_mask = np.asarray(attn_mask)
    W_qkv = np.asarray(W_qkv, dtype=np.float32)
    b_qkv = np.asarray(b_qkv, dtype=np.float32)
    W_proj = np.asarray(W_proj, dtype=np.float32)

    in_maps = []
    for core in range(8):
        b, g = core // 4, core % 4
        s = slice(CL * g, CL * (g + 1))
        wq = W_qkv[:, 0 * C:1 * C][:, s]
        wk = W_qkv[:, 1 * C:2 * C][:, s]
        wv = W_qkv[:, 2 * C:3 * C][:, s]
        bq = b_qkv[0 * C:1 * C][s]
        bk = b_qkv[1 * C:2 * C][s]
        bv = b_qkv[2 * C:3 * C][s]
        bqk = np.concatenate([bq, bk]).reshape(4, P).T  # [128,4], f = fi*128+p
        kbias = (attn_mask[b] != 0).astype(np.float32)  # 0/1 key mask
        in_maps.append({
            "x": np.ascontiguousarray(x[b]),
            "wqk": np.ascontiguousarray(np.concatenate([wq, wk], axis=1)),
            "wv": np.ascontiguousarray(wv),
            "bqk": np.ascontiguousarray(bqk),
            "bv": np.ascontiguousarray(bv.reshape(1, CL)),
            "kbias": np.ascontiguousarray(kbias.reshape(16, P).T),
            "wproj": np.ascontiguousarray(W_proj[s, :]),
        })
    return in_maps


def kernel(x, attn_mask, W_qkv, b_qkv, W_proj, b_proj, _trace=False):
    from concourse.bass_utils import run_bass_kernel_spmd

    nc = _get_nc()
    in_maps = _make_in_maps(x, attn_mask, W_qkv, b_qkv, W_proj)
    res = run_bass_kernel_spmd(nc, in_maps, list(range(8)), trace=_trace)
    outs = res.results

    b_proj = np.asarray(b_proj, dtype=np.float32)
    y = np.empty((2, T, C), dtype=np.float32)
    for b in range(2):
        acc = outs[b * 4]["yT"].T.astype(np.float32).copy()
        for g in range(1, 4):
            acc += outs[b * 4 + g]["yT"].T
        y[b] = acc + b_proj
    if _trace:
        return y, res
    return y


# revision 14
# speedup vs baseline: 1.1685x; 1.0072x over previous
# Causal self-attention on 8 NeuronCores (Trainium2, Bass/Tile).
#
# Problem: B=2, T=2048, C=1024, H=16 heads (hd=64).
#   qkv = x @ W_qkv + b_qkv ; per-head causal softmax attention ; y = att_out @ W_proj + b_proj
#
# Sharding: tensor-parallel over heads x data-parallel over batch.
#   core = b*4 + g   (b in {0,1} batch, g in {0..3} head group of 4 heads)
#   Each core: qkv projection for its 4 heads (W_qkv column shard),
#   attention for those heads, then a partial row-shard projection
#   y_partial^T = W_proj[g-rows]^T @ att_out^T.  Host sums the 4 partials
#   per batch and adds b_proj.
#
# On-chip layout is "transposed" (feature-on-partition) throughout so no
# P-matrix transposes are needed beyond the initial xT build:
#   xT [C, T] (PE transpose) -> qT,kT [64, T] per head, v [T, 64] per head
#   with a PREPENDED ones-column so the PV matmul also produces the softmax
#   denominator on PSUM partition 0 (where gpsimd partition_broadcast can
#   read it directly - no staging DMA), S^T [k, q] chunks (exp on ACT),
#   out^T [65, q] accumulated in PSUM rows 1:65, denom in row 0.
#
# The whole kernel is a single software-pipelined stream: the "phase A"
# work (x transposes, qk/v projections) and the output projection are
# chopped into small thunks that are interleaved between the attention
# S -> exp -> PV steps, so the PE stays busy while ACT chews on exp and
# the attention for q-quad qq starts as soon as k/v for t < (qq+1)*512
# exist.  Matmuls use float32r (full PE rate at N>=256) with fp32 acc.

import numpy as np
from collections import deque

T = 2048
C = 1024
HL = 4          # heads per core
HD = 64
CL = HL * HD    # 256 local channels
P = 128

_cache = {}


def _build_nc():
    import concourse.bass as bass
    import concourse.mybir as mybir
    import concourse.tile as tile
    from concourse import bacc
    from concourse.masks import make_identity
    from contextlib import ExitStack

    f32 = mybir.dt.float32
    f32r = mybir.dt.float32r
    EXP = mybir.ActivationFunctionType.Exp

    nc = bacc.Bacc("TRN2", target_bir_lowering=False)
    x_d = nc.declare_dram_parameter("x", [T, C], f32r, isOutput=False)
    wqk_d = nc.declare_dram_parameter("wqk", [C, 2 * CL], f32r, isOutput=False)
    wv_d = nc.declare_dram_parameter("wv", [C, CL], f32r, isOutput=False)
    bqk_d = nc.declare_dram_parameter("bqk", [P, 4], f32, isOutput=False)
    bv_d = nc.declare_dram_parameter("bv", [1, CL], f32, isOutput=False)
    kbias_d = nc.declare_dram_parameter("kbias", [P, 16], f32, isOutput=False)
    wproj_d = nc.declare_dram_parameter("wproj", [CL, C], f32r, isOutput=False)
    yT_d = nc.declare_dram_parameter("yT", [C, T], f32, isOutput=True)

    NT = T // P       # 16 t-tiles of 128
    NCC = C // P      # 8 contraction chunks of 128
    NQ = T // 512     # 4 q-quads of 512

    with tile.TileContext(nc) as tc, ExitStack() as ctx:
        singles = ctx.enter_context(tc.tile_pool(name="singles", bufs=1))

        # persistent SBUF
        qkT = singles.tile([P, 4, T], f32r)         # rows: [q f0,q f1,k f0,k f1]
        vv = singles.tile([P, NT, HL, HD + 1], f32r)  # ones col FIRST, then v
        AT = singles.tile([P, 2, T], f32r)          # attention out^T (c' x t)
        xT = singles.tile([P, NCC, T], f32r)
        wqk_sb = singles.tile([P, NCC, 2 * CL], f32r)
        wv_sb = singles.tile([P, NCC, CL], f32r)
        wproj_sb = singles.tile([P, 2, C], f32r)
        tri01 = singles.tile([P, P], f32)          # lower-tri 1.0 / 0.0
        kbias_sb = singles.tile([P, 16], f32)
        bqk_sb = singles.tile([P, 4], f32)
        bv_sb = singles.tile([P, HL, HD], f32)
        identr = singles.tile([P, P], f32r)

        make_identity(nc, identr.bitcast(f32))
        # tri01[k, q] = 1.0 where q >= k else 0.0
        nc.gpsimd.memset(tri01, 1.0)
        nc.gpsimd.affine_select(
            out=tri01,
            in_=tri01,
            compare_op=mybir.AluOpType.is_ge,
            fill=0.0,
            base=0,
            pattern=[[1, P]],
            channel_multiplier=-1,
        )

        # small/params on the gpsimd (SWDGE) queue so the big x loads on
        # the sync queue aren't delayed
        nc.scalar.dma_start(out=kbias_sb, in_=kbias_d[:])
        nc.scalar.dma_start(out=bqk_sb, in_=bqk_d[:])
        nc.scalar.dma_start(
            out=bv_sb,
            in_=bv_d[:].rearrange("o (h d) -> o h d", h=HL).to_broadcast([P, HL, HD]),
        )
        nc.vector.memset(vv[:, :, :, 0].bitcast(f32), 1.0)

        # weights load in chunks as filler thunks (unit_W* below) so the
        # gpsimd queue doesn't serialize 12.6us of weight DMA at startup

        with (
            tc.tile_pool(name="xst", bufs=2) as xst,
            tc.tile_pool(name="pa_ps", bufs=2, space="PSUM") as pa_ps,
            tc.tile_pool(name="ps_s", bufs=2, space="PSUM") as ps_s,
            tc.tile_pool(name="ps_o", bufs=2, space="PSUM") as ps_o,
            tc.tile_pool(name="ptp", bufs=3) as ptp,
            tc.tile_pool(name="ep", bufs=2) as ep,
            tc.tile_pool(name="yst", bufs=3) as yst,
        ):
            # ---- filler thunk machinery ------------------------------------
            # Units of "phase A" / projection work are chopped into small
            # thunks tagged with the quad index that requires their data.
            # flush_stage(s) runs everything needed before quad s; emit_some
            # interleaves thunks into the attention stream as PE filler.
            FQ = deque()

            def emit_some(n):
                for _ in range(n):
                    if not FQ:
                        return
                    FQ.popleft()[1]()

            def flush_stage(s):
                while FQ and FQ[0][0] <= s:
                    FQ.popleft()[1]()

            def flush_all():
                while FQ:
                    FQ.popleft()[1]()

            def unit_Wqk(fi, stage):
                def w():
                    nc.gpsimd.dma_start(
                        out=wqk_sb[:, :, fi * P:(fi + 1) * P],
                        in_=wqk_d[:, fi * P:(fi + 1) * P].rearrange(
                            "(o p) n -> p o n", p=P),
                    )

                FQ.append((stage, w))

            def unit_Wv(half, stage):
                def w():
                    nc.gpsimd.dma_start(
                        out=wv_sb[:, half * 4:(half + 1) * 4, :],
                        in_=wv_d[half * 512:(half + 1) * 512, :].rearrange(
                            "(o p) n -> p o n", p=P),
                    )

                FQ.append((stage, w))

            def unit_Wproj(cc, stage):
                def w():
                    nc.gpsimd.dma_start(
                        out=wproj_sb[:, cc, :],
                        in_=wproj_d[cc * P:(cc + 1) * P, :].rearrange(
                            "(o p) n -> p o n", p=P),
                    )

                FQ.append((stage, w))

            def unit_T(ti, stage):
                # load x tile ti and build xT[:, :, ti*P:(ti+1)*P]
                st = {}

                def t_load():
                    st["xt"] = xst.tile([P, C], f32r, name="xt")
                    # early tiles alternate between the sync and scalar
                    # HWDGE queues so the serial load latency halves
                    deng = nc.scalar if (ti < 4 and ti % 2 == 0) else nc.sync
                    deng.dma_start(out=st["xt"], in_=x_d[ti * P:(ti + 1) * P, :])

                FQ.append((stage, t_load))
                for half in (0, 1):
                    def t_tr(half=half):
                        pt = pa_ps.tile([P, 4, P], f32r, tag="pa")
                        for m in range(4):
                            ci = half * 4 + m
                            nc.tensor.transpose(
                                pt[:, m, :], st["xt"][:, ci * P:(ci + 1) * P], identr
                            )
                        eng = nc.vector if (ti < 4 or (ti + half) % 2 == 0) \
                            else nc.gpsimd
                        eng.tensor_copy(
                            out=xT[:, half * 4:(half + 1) * 4, ti * P:(ti + 1) * P],
                            in_=pt,
                        )

                    FQ.append((stage, t_tr))

            def unit_QK(fi, tj, stage):
                # qkT[:, fi, tj*512:(tj+1)*512] = W chunk^T @ xT + bias
                st = {}

                def q_first():
                    st["pq"] = pa_ps.tile([P, 512], f32, tag="pa", name="pq")
                    for ci in range(2):
                        nc.tensor.matmul(
                            st["pq"],
                            lhsT=wqk_sb[:, ci, fi * P:(fi + 1) * P],
                            rhs=xT[:, ci, tj * 512:(tj + 1) * 512],
                            start=(ci == 0),
                            stop=False,
                        )

                FQ.append((stage, q_first))
                for cb in (2, 4, 6):
                    def q_mid(cb=cb):
                        for ci in range(cb, cb + 2):
                            nc.tensor.matmul(
                                st["pq"],
                                lhsT=wqk_sb[:, ci, fi * P:(fi + 1) * P],
                                rhs=xT[:, ci, tj * 512:(tj + 1) * 512],
                                start=False,
                                stop=(ci == NCC - 1),
                            )

                    FQ.append((stage, q_mid))

                def q_bias():
                    eng = nc.vector if fi % 2 == 0 else nc.gpsimd
                    eng.tensor_scalar_add(
                        out=qkT[:, fi, tj * 512:(tj + 1) * 512],
                        in0=st["pq"],
                        scalar1=bqk_sb[:, fi:fi + 1],
                    )

                FQ.append((stage, q_bias))

            def unit_V(tp, stage):
                # v for t-tiles 2tp, 2tp+1 -> vv[:, ti, :, 1:65]
                st = {}

                def v_first():
                    st["pv"] = pa_ps.tile([P, 2, CL], f32, tag="pa", name="pv")
                    for ci in range(4):
                        nc.tensor.matmul(
                            st["pv"][:, 0, :],
                            lhsT=xT[:, ci, 2 * tp * P:(2 * tp + 1) * P],
                            rhs=wv_sb[:, ci, :],
                            start=(ci == 0),
                            stop=False,
                        )

                FQ.append((stage, v_first))

                def v_second():
                    for ci in range(4, NCC):
                        nc.tensor.matmul(
                            st["pv"][:, 0, :],
                            lhsT=xT[:, ci, 2 * tp * P:(2 * tp + 1) * P],
                            rhs=wv_sb[:, ci, :],
                            start=False,
                            stop=(ci == NCC - 1),
                        )

                FQ.append((stage, v_second))

                def v_third():
                    for ci in range(4):
                        nc.tensor.matmul(
                            st["pv"][:, 1, :],
                            lhsT=xT[:, ci, (2 * tp + 1) * P:(2 * tp + 2) * P],
                            rhs=wv_sb[:, ci, :],
                            start=(ci == 0),
                            stop=False,
                        )

                FQ.append((stage, v_third))

                def v_fourth():
                    for ci in range(4, NCC):
                        nc.tensor.matmul(
                            st["pv"][:, 1, :],
                            lhsT=xT[:, ci, (2 * tp + 1) * P:(2 * tp + 2) * P],
                            rhs=wv_sb[:, ci, :],
                            start=False,
                            stop=(ci == NCC - 1),
                        )

                FQ.append((stage, v_fourth))

                def v_bias(k, ti):
                    eng = nc.vector if k == 0 else nc.gpsimd
                    eng.tensor_add(
                        out=vv[:, ti, :, 1:HD + 1],
                        in0=st["pv"][:, k, :].rearrange("p (h d) -> p h d", h=HL),
                        in1=bv_sb,
                    )
                    # key-padding mask: zero this key's v row AND its ones-col
                    # entry (excludes it from numerator and denominator)
                    eng.tensor_scalar_mul(
                        out=vv[:, ti, :, :],
                        in0=vv[:, ti, :, :],
                        scalar1=kbias_sb[:, ti:ti + 1],
                    )

                FQ.append((stage, lambda: v_bias(0, 2 * tp)))
                FQ.append((stage, lambda: v_bias(1, 2 * tp + 1)))

            def unit_PR(qq, stage):
                # projection for quad qq: yT[:, qq*512:(qq+1)*512]
                for co in range(C // P):
                    def pr(co=co):
                        py = pa_ps.tile([P, 512], f32, tag="pa")
                        for cc in range(2):
                            nc.tensor.matmul(
                                py,
                                lhsT=wproj_sb[:, cc, co * P:(co + 1) * P],
                                rhs=AT[:, cc, qq * 512:(qq + 1) * 512],
                                start=(cc == 0),
                                stop=(cc == 1),
                            )
                        yt = yst.tile([P, 512], f32)
                        if co % 3 == 0:
                            nc.vector.tensor_copy(out=yt, in_=py)
                        elif co % 3 == 1:
                            nc.gpsimd.tensor_copy(out=yt, in_=py)
                        else:
                            nc.scalar.copy(out=yt, in_=py)
                        deng = nc.sync if co % 2 == 0 else nc.gpsimd
                        deng.dma_start(
                            out=yT_d[co * P:(co + 1) * P, qq * 512:(qq + 1) * 512],
                            in_=yt,
                        )

                    FQ.append((stage, pr))

            # ---- attention for one (quad, head) ----------------------------
            def attn(qq, h):
                nf = max(1, 3 - qq)   # filler pops per site; deeper early
                bp = (h % 2) * HD
                fo = h // 2
                qTh = qkT[bp:bp + HD, fo, :]
                kTh = qkT[bp:bp + HD, 2 + fo, :]
                po = ps_o.tile([HD + 1, 512], f32)
                qs = qq * 512
                # full (below-diagonal) chunk PAIRS
                for jp in range(2 * qq):
                    j0 = 2 * jp
                    ps2 = ps_s.tile([P, 2, 512], f32, tag="s")
                    for m in range(2):
                        nc.tensor.matmul(
                            ps2[:, m, :],
                            lhsT=kTh[:, (j0 + m) * P:(j0 + m + 1) * P],
                            rhs=qTh[:, qs:qs + 512],
                            start=True,
                            stop=True,
                        )
                    pT2 = ptp.tile([P, 2, 512], f32r, tag="p")
                    nc.scalar.activation(out=pT2, in_=ps2, func=EXP, scale=0.125)
                    emit_some(nf)
                    for m in range(2):
                        nc.tensor.matmul(
                            po,
                            lhsT=vv[:, j0 + m, h, :],
                            rhs=pT2[:, m, :],
                            start=(j0 + m == 0),
                            stop=False,
                        )
                    emit_some(nf)
                # diagonal-region chunks o=0..3 (keys jb..jb+3), packed into
                # two ps_s tiles so the exp batches:
                #   tile A: o0 @ [0:512] (q 0:512), o1 @ [512:896] (q 128:512)
                #   tile B: o2 @ [0:256] (q 256:512), o3 @ [256:512] (q 256:512)
                jb = 4 * qq
                A = ps_s.tile([P, 2, 512], f32, tag="s")
                Af = A.rearrange("p a b -> p (a b)")
                nc.tensor.matmul(
                    A[:, 0, :], lhsT=kTh[:, jb * P:(jb + 1) * P],
                    rhs=qTh[:, qs:qs + 512], start=True, stop=True,
                )
                nc.tensor.matmul(
                    Af[:, 512:896], lhsT=kTh[:, (jb + 1) * P:(jb + 2) * P],
                    rhs=qTh[:, qs + 128:qs + 512], start=True, stop=True,
                )
                pTA = ptp.tile([P, 2, 512], f32r, tag="p")
                pTAf = pTA.rearrange("p a b -> p (a b)")
                nc.scalar.activation(
                    out=pTAf[:, 0:896], in_=Af[:, 0:896], func=EXP, scale=0.125,
                )
                emit_some(nf)
                # causal tri-mask on the first 128 cols of each diag block
                nc.vector.tensor_mul(
                    out=pTA[:, 0, 0:P], in0=pTA[:, 0, 0:P], in1=tri01,
                )
                nc.gpsimd.tensor_mul(
                    out=pTAf[:, 512:512 + P], in0=pTAf[:, 512:512 + P], in1=tri01,
                )
                nc.tensor.matmul(
                    po, lhsT=vv[:, jb, h, :], rhs=pTA[:, 0, :],
                    start=(jb == 0), stop=False,
                )
                nc.tensor.matmul(
                    po[:, 128:], lhsT=vv[:, jb + 1, h, :], rhs=pTAf[:, 512:896],
                    start=False, stop=False,
                )
                B = ps_s.tile([P, 2, 512], f32, tag="s")
                nc.tensor.matmul(
                    B[:, 0, 0:256], lhsT=kTh[:, (jb + 2) * P:(jb + 3) * P],
                    rhs=qTh[:, qs + 256:qs + 512], start=True, stop=True,
                )
                nc.tensor.matmul(
                    B[:, 0, 256:512], lhsT=kTh[:, (jb + 3) * P:(jb + 4) * P],
                    rhs=qTh[:, qs + 256:qs + 512], start=True, stop=True,
                )
                pTB = ptp.tile([P, 2, 512], f32r, tag="p")
                nc.scalar.activation(
                    out=pTB[:, 0, :], in_=B[:, 0, :], func=EXP, scale=0.125,
                )
                emit_some(nf)
                # o3 cols [256:384] (q 256:384 vs keys >= 384) are causally
                # invalid; zero them so the padded-width PV adds nothing
                nc.vector.memset(pTB[:, 0, 256:384].bitcast(f32), 0.0)
                nc.vector.tensor_mul(
                    out=pTB[:, 0, 0:P], in0=pTB[:, 0, 0:P], in1=tri01,
                )
                nc.gpsimd.tensor_mul(
                    out=pTB[:, 0, 384:512], in0=pTB[:, 0, 384:512], in1=tri01,
                )
                nc.tensor.matmul(
                    po[:, 256:], lhsT=vv[:, jb + 2, h, :], rhs=pTB[:, 0, 0:256],
                    start=False, stop=False,
                )
                nc.tensor.matmul(
                    po[:, 256:], lhsT=vv[:, jb + 3, h, :], rhs=pTB[:, 0, 256:512],
                    start=False, stop=True,
                )
                # normalize: rows 1:65 divided by row 0 (the ones-col sum).
                # The denom lives on PSUM partition 0, so partition_broadcast
                # can fan out its reciprocal without any staging DMA.  The
                # mul covers the aligned rows 0:65 (row 0 becomes den/den=1,
                # harmless); the DMA then ships rows 1:65 into AT.
                rcp = ep.tile([1, 512], f32r, tag="rcp")
                with nc.allow_low_precision(
                    reason="f32r reciprocal of softmax denom; 2^-11 rel"
                ):
                    nc.vector.reciprocal(out=rcp, in_=po[0:1, :])
                rb = ep.tile([HD + 1, 512], f32r, tag="rb")
                nc.gpsimd.partition_broadcast(rb, rcp)
                ob = ep.tile([HD + 1, 512], f32r, tag="ob")
                meng = nc.vector if h % 2 == 0 else nc.gpsimd
                meng.tensor_mul(out=ob, in0=po, in1=rb)
                deng = nc.sync if h % 2 == 0 or qq == NQ - 1 else nc.gpsimd
                deng.dma_start(
                    out=AT[bp:bp + HD, fo, qs:qs + 512], in_=ob[1:HD + 1, :],
                )
                emit_some(2 * nf)

            # ---- schedule --------------------------------------------------
            # preload the ACT exp table (~1.3us) while the PE warms up
            warm = ep.tile([1, 8], f32r, tag="rcp", name="warm")
            nc.scalar.activation(out=warm, in_=tri01[0:1, 0:8], func=EXP)

            # stage-0 prologue (everything quad 0 needs); weight chunks
            # interleave with the x loads/transposes
            unit_Wqk(0, 0)
            unit_T(0, 0)
            unit_Wqk(2, 0)
            unit_T(1, 0)
            unit_Wv(0, 0)
            unit_T(2, 0)
            unit_Wv(1, 0)
            unit_T(3, 0)
            unit_QK(0, 0, 0)
            unit_QK(2, 0, 0)
            unit_V(0, 0)
            unit_Wqk(1, 0)
            unit_V(1, 0)
            unit_Wqk(3, 0)
            unit_QK(1, 0, 0)
            unit_QK(3, 0, 0)
            flush_stage(0)

            for qq in range(NQ):
                if qq < NQ - 1:
                    s = qq + 1
                    for ti in range(4 * s, 4 * s + 4):
                        unit_T(ti, s)
                    if s == 1:
                        unit_Wproj(0, s)
                    unit_QK(0, s, s)
                    unit_QK(2, s, s)
                    if s == 1:
                        unit_Wproj(1, s)
                    unit_V(2 * s, s)
                    unit_QK(1, s, s)
                    unit_V(2 * s + 1, s)
                    unit_QK(3, s, s)
                if 1 <= qq <= 2:
                    # projection of the previous quad interleaves as filler
                    # into this quad and must flush before quad qq+1 ends
                    unit_PR(qq - 1, qq + 2)
                for h in range(HL):
                    attn(qq, h)
                if qq == NQ - 1:
                    # PR(2) emits after the last quad's heads: its matmuls
                    # keep the PE busy while the final head's normalize
                    # chain completes, so PR(3) then flows without a stall
                    unit_PR(qq - 1, 99)
                if qq < NQ - 1:
                    flush_stage(qq + 1)
            flush_all()
            unit_PR(NQ - 1, 99)
            flush_all()

    return nc


def _get_nc():
    if "nc" not in _cache:
        nc = _build_nc()
        nc.finalize()  # runs the Bacc pass pipeline (reg alloc, wait splitting)
        _cache["nc"] = nc
    return _cache["nc"]


def _make_in_maps(x, attn_mask, W_qkv, b_qkv, W_proj):
    x = np.asarray(x, dtype=np.float32)
    attn_mask = np.asarray(attn_mask)
    W_qkv = np.asarray(W_qkv, dtype=np.float32)
    b_qkv = np.asarray(b_qkv, dtype=np.float32)
    W_proj = np.asarray(W_proj, dtype=np.float32)

    in_maps = []
    for core in range(8):
        b, g = core // 4, core % 4
        s = slice(CL * g, CL * (g + 1))
        wq = W_qkv[:, 0 * C:1 * C][:, s]
        wk = W_qkv[:, 1 * C:2 * C][:, s]
        wv = W_qkv[:, 2 * C:3 * C][:, s]
        bq = b_qkv[0 * C:1 * C][s]
        bk = b_qkv[1 * C:2 * C][s]
        bv = b_qkv[2 * C:3 * C][s]
        bqk = np.concatenate([bq, bk]).reshape(4, P).T  # [128,4], f = fi*128+p
        kbias = (attn_mask[b] != 0).astype(np.float32)  # 0/1 key mask
        in_maps.append({
            "x": np.ascontiguousarray(x[b]),
            "wqk": np.ascontiguousarray(np.concatenate([wq, wk], axis=1)),
            "wv": np.ascontiguousarray(wv),
            "bqk": np.ascontiguousarray(bqk),
            "bv": np.ascontiguousarray(bv.reshape(1, CL)),
            "kbias": np.ascontiguousarray(kbias.reshape(16, P).T),
            "wproj": np.ascontiguousarray(W_proj[s, :]),
        })
    return in_maps


def kernel(x, attn_mask, W_qkv, b_qkv, W_proj, b_proj, _trace=False):
    from concourse.bass_utils import run_bass_kernel_spmd

    nc = _get_nc()
    in_maps = _make_in_maps(x, attn_mask, W_qkv, b_qkv, W_proj)
    res = run_bass_kernel_spmd(nc, in_maps, list(range(8)), trace=_trace)
    outs = res.results

    b_proj = np.asarray(b_proj, dtype=np.float32)
    y = np.empty((2, T, C), dtype=np.float32)
    for b in range(2):
        acc = outs[b * 4]["yT"].T.astype(np.float32).copy()
        for g in range(1, 4):
            acc += outs[b * 4 + g]["yT"].T
        y[b] = acc + b_proj
    if _trace:
        return y, res
    return y


# revision 15
# speedup vs baseline: 1.1822x; 1.0117x over previous
# Causal self-attention on 8 NeuronCores (Trainium2, Bass/Tile).
#
# Problem: B=2, T=2048, C=1024, H=16 heads (hd=64).
#   qkv = x @ W_qkv + b_qkv ; per-head causal softmax attention ; y = att_out @ W_proj + b_proj
#
# Sharding: tensor-parallel over heads x data-parallel over batch.
#   core = b*4 + g   (b in {0,1} batch, g in {0..3} head group of 4 heads)
#   Each core: qkv projection for its 4 heads (W_qkv column shard),
#   attention for those heads, then a partial row-shard projection
#   y_partial^T = W_proj[g-rows]^T @ att_out^T.  Host sums the 4 partials
#   per batch and adds b_proj.
#
# On-chip layout is "transposed" (feature-on-partition) throughout so no
# P-matrix transposes are needed beyond the initial xT build:
#   xT [C, T] (PE transpose) -> qT,kT [64, T] per head, v [T, 64] per head
#   with a PREPENDED ones-column so the PV matmul also produces the softmax
#   denominator on PSUM partition 0 (where gpsimd partition_broadcast can
#   read it directly - no staging DMA), S^T [k, q] chunks (exp on ACT),
#   out^T [65, q] accumulated in PSUM rows 1:65, denom in row 0.
#
# The whole kernel is a single software-pipelined stream: the "phase A"
# work (x transposes, qk/v projections) and the output projection are
# chopped into small thunks that are interleaved between the attention
# S -> exp -> PV steps, so the PE stays busy while ACT chews on exp and
# the attention for q-quad qq starts as soon as k/v for t < (qq+1)*512
# exist.  Matmuls use float32r (full PE rate at N>=256) with fp32 acc.

import numpy as np
from collections import deque

T = 2048
C = 1024
HL = 4          # heads per core
HD = 64
CL = HL * HD    # 256 local channels
P = 128

_cache = {}


def _build_nc():
    import concourse.bass as bass
    import concourse.mybir as mybir
    import concourse.tile as tile
    from concourse import bacc
    from concourse.masks import make_identity
    from contextlib import ExitStack

    f32 = mybir.dt.float32
    f32r = mybir.dt.float32r
    EXP = mybir.ActivationFunctionType.Exp

    nc = bacc.Bacc("TRN2", target_bir_lowering=False)
    x_d = nc.declare_dram_parameter("x", [T, C], f32r, isOutput=False)
    wqk_d = nc.declare_dram_parameter("wqk", [C, 2 * CL], f32r, isOutput=False)
    wv_d = nc.declare_dram_parameter("wv", [C, CL], f32r, isOutput=False)
    bqk_d = nc.declare_dram_parameter("bqk", [P, 4], f32, isOutput=False)
    bv_d = nc.declare_dram_parameter("bv", [1, CL], f32, isOutput=False)
    kbias_d = nc.declare_dram_parameter("kbias", [P, 16], f32, isOutput=False)
    wproj_d = nc.declare_dram_parameter("wproj", [CL, C], f32r, isOutput=False)
    yT_d = nc.declare_dram_parameter("yT", [C, T], f32, isOutput=True)

    NT = T // P       # 16 t-tiles of 128
    NCC = C // P      # 8 contraction chunks of 128
    NQ = T // 512     # 4 q-quads of 512

    with tile.TileContext(nc) as tc, ExitStack() as ctx:
        singles = ctx.enter_context(tc.tile_pool(name="singles", bufs=1))

        # persistent SBUF
        qkT = singles.tile([P, 4, T], f32r)         # rows: [q f0,q f1,k f0,k f1]
        vv = singles.tile([P, NT, HL, HD + 1], f32r)  # ones col FIRST, then v
        # attention out^T (c' x t), one tile per q-quad so projection
        # reads never false-depend on a later quad's writes
        ATq = [singles.tile([P, 2, 512], f32r, name=f"AT{i}") for i in range(4)]
        xT = singles.tile([P, NCC, T], f32r)
        wqk_sb = singles.tile([P, NCC, 2 * CL], f32r)
        wv_sb = singles.tile([P, NCC, CL], f32r)
        wproj_sb = singles.tile([P, 2, C], f32r)
        tri01 = singles.tile([P, P], f32)          # lower-tri 1.0 / 0.0
        kbias_sb = singles.tile([P, 16], f32)
        bqk_sb = singles.tile([P, 4], f32)
        bv_sb = singles.tile([P, HL, HD], f32)
        identr = singles.tile([P, P], f32r)

        make_identity(nc, identr.bitcast(f32))
        # tri01[k, q] = 1.0 where q >= k else 0.0
        nc.gpsimd.memset(tri01, 1.0)
        nc.gpsimd.affine_select(
            out=tri01,
            in_=tri01,
            compare_op=mybir.AluOpType.is_ge,
            fill=0.0,
            base=0,
            pattern=[[1, P]],
            channel_multiplier=-1,
        )

        nc.vector.memset(vv[:, :, :, 0].bitcast(f32), 1.0)

        # weights load in chunks as filler thunks (unit_W* below) so the
        # gpsimd queue doesn't serialize 12.6us of weight DMA at startup

        with (
            tc.tile_pool(name="xst", bufs=2) as xst,
            tc.tile_pool(name="pa_ps", bufs=2, space="PSUM") as pa_ps,
            tc.tile_pool(name="ps_s", bufs=2, space="PSUM") as ps_s,
            tc.tile_pool(name="ps_o", bufs=2, space="PSUM") as ps_o,
            tc.tile_pool(name="ptp", bufs=3) as ptp,
            tc.tile_pool(name="ep", bufs=2) as ep,
            tc.tile_pool(name="yst", bufs=3) as yst,
        ):
            # ---- filler thunk machinery ------------------------------------
            # Units of "phase A" / projection work are chopped into small
            # thunks tagged with the quad index that requires their data.
            # flush_stage(s) runs everything needed before quad s; emit_some
            # interleaves thunks into the attention stream as PE filler.
            FQ = deque()

            def emit_some(n):
                for _ in range(n):
                    if not FQ:
                        return
                    FQ.popleft()[1]()

            def flush_stage(s):
                while FQ and FQ[0][0] <= s:
                    FQ.popleft()[1]()

            def flush_all():
                while FQ:
                    FQ.popleft()[1]()

            def unit_Wqk(fi, stage):
                def w():
                    nc.gpsimd.dma_start(
                        out=wqk_sb[:, :, fi * P:(fi + 1) * P],
                        in_=wqk_d[:, fi * P:(fi + 1) * P].rearrange(
                            "(o p) n -> p o n", p=P),
                    )

                FQ.append((stage, w))

            def unit_Wv(half, stage):
                def w():
                    nc.gpsimd.dma_start(
                        out=wv_sb[:, half * 4:(half + 1) * 4, :],
                        in_=wv_d[half * 512:(half + 1) * 512, :].rearrange(
                            "(o p) n -> p o n", p=P),
                    )

                FQ.append((stage, w))

            def unit_Wproj(cc, stage):
                def w():
                    nc.gpsimd.dma_start(
                        out=wproj_sb[:, cc, :],
                        in_=wproj_d[cc * P:(cc + 1) * P, :].rearrange(
                            "(o p) n -> p o n", p=P),
                    )

                FQ.append((stage, w))

            def unit_T(ti, stage):
                # load x tile ti and build xT[:, :, ti*P:(ti+1)*P]
                st = {}

                def t_load():
                    st["xt"] = xst.tile([P, C], f32r, name="xt")
                    # early tiles alternate between the sync and scalar
                    # HWDGE queues so the serial load latency halves
                    deng = nc.scalar if (ti < 4 and ti % 2 == 0) else nc.sync
                    deng.dma_start(out=st["xt"], in_=x_d[ti * P:(ti + 1) * P, :])

                FQ.append((stage, t_load))
                for half in (0, 1):
                    def t_tr(half=half):
                        pt = pa_ps.tile([P, 4, P], f32r, tag="pa")
                        for m in range(4):
                            ci = half * 4 + m
                            nc.tensor.transpose(
                                pt[:, m, :], st["xt"][:, ci * P:(ci + 1) * P], identr
                            )
                        eng = nc.vector if (ti < 4 or (ti + half) % 2 == 0) \
                            else nc.gpsimd
                        eng.tensor_copy(
                            out=xT[:, half * 4:(half + 1) * 4, ti * P:(ti + 1) * P],
                            in_=pt,
                        )

                    FQ.append((stage, t_tr))

            def unit_QK(fi, tj, stage):
                # qkT[:, fi, tj*512:(tj+1)*512] = W chunk^T @ xT + bias
                st = {}

                def q_first():
                    st["pq"] = pa_ps.tile([P, 512], f32, tag="pa", name="pq")
                    for ci in range(2):
                        nc.tensor.matmul(
                            st["pq"],
                            lhsT=wqk_sb[:, ci, fi * P:(fi + 1) * P],
                            rhs=xT[:, ci, tj * 512:(tj + 1) * 512],
                            start=(ci == 0),
                            stop=False,
                        )

                FQ.append((stage, q_first))
                for cb in (2, 4, 6):
                    def q_mid(cb=cb):
                        for ci in range(cb, cb + 2):
                            nc.tensor.matmul(
                                st["pq"],
                                lhsT=wqk_sb[:, ci, fi * P:(fi + 1) * P],
                                rhs=xT[:, ci, tj * 512:(tj + 1) * 512],
                                start=False,
                                stop=(ci == NCC - 1),
                            )

                    FQ.append((stage, q_mid))

                def q_bias():
                    eng = nc.vector if fi % 2 == 0 else nc.gpsimd
                    eng.tensor_scalar_add(
                        out=qkT[:, fi, tj * 512:(tj + 1) * 512],
                        in0=st["pq"],
                        scalar1=bqk_sb[:, fi:fi + 1],
                    )

                FQ.append((stage, q_bias))

            def unit_V(tp, stage):
                # v for t-tiles 2tp, 2tp+1 -> vv[:, ti, :, 1:65]
                st = {}

                def v_first():
                    st["pv"] = pa_ps.tile([P, 2, CL], f32, tag="pa", name="pv")
                    for ci in range(4):
                        nc.tensor.matmul(
                            st["pv"][:, 0, :],
                            lhsT=xT[:, ci, 2 * tp * P:(2 * tp + 1) * P],
                            rhs=wv_sb[:, ci, :],
                            start=(ci == 0),
                            stop=False,
                        )

                FQ.append((stage, v_first))

                def v_second():
                    for ci in range(4, NCC):
                        nc.tensor.matmul(
                            st["pv"][:, 0, :],
                            lhsT=xT[:, ci, 2 * tp * P:(2 * tp + 1) * P],
                            rhs=wv_sb[:, ci, :],
                            start=False,
                            stop=(ci == NCC - 1),
                        )

                FQ.append((stage, v_second))

                def v_third():
                    for ci in range(4):
                        nc.tensor.matmul(
                            st["pv"][:, 1, :],
                            lhsT=xT[:, ci, (2 * tp + 1) * P:(2 * tp + 2) * P],
                            rhs=wv_sb[:, ci, :],
                            start=(ci == 0),
                            stop=False,
                        )

                FQ.append((stage, v_third))

                def v_fourth():
                    for ci in range(4, NCC):
                        nc.tensor.matmul(
                            st["pv"][:, 1, :],
                            lhsT=xT[:, ci, (2 * tp + 1) * P:(2 * tp + 2) * P],
                            rhs=wv_sb[:, ci, :],
                            start=False,
                            stop=(ci == NCC - 1),
                        )

                FQ.append((stage, v_fourth))

                def v_bias(k, ti):
                    eng = nc.vector if k == 0 else nc.gpsimd
                    eng.tensor_add(
                        out=vv[:, ti, :, 1:HD + 1],
                        in0=st["pv"][:, k, :].rearrange("p (h d) -> p h d", h=HL),
                        in1=bv_sb,
                    )
                    # key-padding mask: zero this key's v row AND its ones-col
                    # entry (excludes it from numerator and denominator)
                    eng.tensor_scalar_mul(
                        out=vv[:, ti, :, :],
                        in0=vv[:, ti, :, :],
                        scalar1=kbias_sb[:, ti:ti + 1],
                    )

                FQ.append((stage, lambda: v_bias(0, 2 * tp)))
                FQ.append((stage, lambda: v_bias(1, 2 * tp + 1)))

            def unit_PR(qq, stage, use_act=False):
                # projection for quad qq: yT[:, qq*512:(qq+1)*512]
                for co in range(C // P):
                    def pr(co=co):
                        py = pa_ps.tile([P, 512], f32, tag="pa")
                        for cc in range(2):
                            nc.tensor.matmul(
                                py,
                                lhsT=wproj_sb[:, cc, co * P:(co + 1) * P],
                                rhs=ATq[qq][:, cc, :],
                                start=(cc == 0),
                                stop=(cc == 1),
                            )
                        yt = yst.tile([P, 512], f32)
                        if use_act and co % 3 == 2:
                            nc.scalar.copy(out=yt, in_=py)
                        elif co % 2 == 0:
                            nc.vector.tensor_copy(out=yt, in_=py)
                        else:
                            nc.gpsimd.tensor_copy(out=yt, in_=py)
                        deng = nc.sync if co % 2 == 0 else nc.gpsimd
                        deng.dma_start(
                            out=yT_d[co * P:(co + 1) * P, qq * 512:(qq + 1) * 512],
                            in_=yt,
                        )

                    FQ.append((stage, pr))

            # ---- attention for one (quad, head) ----------------------------
            def attn(qq, h):
                nf = max(1, 3 - qq)   # filler pops per site; deeper early
                bp = (h % 2) * HD
                fo = h // 2
                qTh = qkT[bp:bp + HD, fo, :]
                kTh = qkT[bp:bp + HD, 2 + fo, :]
                po = ps_o.tile([HD + 1, 512], f32)
                qs = qq * 512
                # full (below-diagonal) chunk PAIRS
                for jp in range(2 * qq):
                    j0 = 2 * jp
                    ps2 = ps_s.tile([P, 2, 512], f32, tag="s")
                    for m in range(2):
                        nc.tensor.matmul(
                            ps2[:, m, :],
                            lhsT=kTh[:, (j0 + m) * P:(j0 + m + 1) * P],
                            rhs=qTh[:, qs:qs + 512],
                            start=True,
                            stop=True,
                        )
                    pT2 = ptp.tile([P, 2, 512], f32r, tag="p")
                    nc.scalar.activation(out=pT2, in_=ps2, func=EXP, scale=0.125)
                    emit_some(nf)
                    for m in range(2):
                        nc.tensor.matmul(
                            po,
                            lhsT=vv[:, j0 + m, h, :],
                            rhs=pT2[:, m, :],
                            start=(j0 + m == 0),
                            stop=False,
                        )
                    emit_some(nf)
                # diagonal-region chunks o=0..3 (keys jb..jb+3), packed into
                # two ps_s tiles so the exp batches:
                #   tile A: o0 @ [0:512] (q 0:512), o1 @ [512:896] (q 128:512)
                #   tile B: o2 @ [0:256] (q 256:512), o3 @ [256:512] (q 256:512)
                jb = 4 * qq
                A = ps_s.tile([P, 2, 512], f32, tag="s")
                Af = A.rearrange("p a b -> p (a b)")
                nc.tensor.matmul(
                    A[:, 0, :], lhsT=kTh[:, jb * P:(jb + 1) * P],
                    rhs=qTh[:, qs:qs + 512], start=True, stop=True,
                )
                nc.tensor.matmul(
                    Af[:, 512:896], lhsT=kTh[:, (jb + 1) * P:(jb + 2) * P],
                    rhs=qTh[:, qs + 128:qs + 512], start=True, stop=True,
                )
                pTA = ptp.tile([P, 2, 512], f32r, tag="p")
                pTAf = pTA.rearrange("p a b -> p (a b)")
                nc.scalar.activation(
                    out=pTAf[:, 0:896], in_=Af[:, 0:896], func=EXP, scale=0.125,
                )
                emit_some(nf)
                # causal tri-mask on the first 128 cols of each diag block
                nc.vector.tensor_mul(
                    out=pTA[:, 0, 0:P], in0=pTA[:, 0, 0:P], in1=tri01,
                )
                nc.gpsimd.tensor_mul(
                    out=pTAf[:, 512:512 + P], in0=pTAf[:, 512:512 + P], in1=tri01,
                )
                nc.tensor.matmul(
                    po, lhsT=vv[:, jb, h, :], rhs=pTA[:, 0, :],
                    start=(jb == 0), stop=False,
                )
                nc.tensor.matmul(
                    po[:, 128:], lhsT=vv[:, jb + 1, h, :], rhs=pTAf[:, 512:896],
                    start=False, stop=False,
                )
                B = ps_s.tile([P, 2, 512], f32, tag="s")
                nc.tensor.matmul(
                    B[:, 0, 0:256], lhsT=kTh[:, (jb + 2) * P:(jb + 3) * P],
                    rhs=qTh[:, qs + 256:qs + 512], start=True, stop=True,
                )
                nc.tensor.matmul(
                    B[:, 0, 256:512], lhsT=kTh[:, (jb + 3) * P:(jb + 4) * P],
                    rhs=qTh[:, qs + 256:qs + 512], start=True, stop=True,
                )
                pTB = ptp.tile([P, 2, 512], f32r, tag="p")
                nc.scalar.activation(
                    out=pTB[:, 0, :], in_=B[:, 0, :], func=EXP, scale=0.125,
                )
                emit_some(nf)
                # o3 cols [256:384] (q 256:384 vs keys >= 384) are causally
                # invalid; zero them so the padded-width PV adds nothing
                nc.vector.memset(pTB[:, 0, 256:384].bitcast(f32), 0.0)
                nc.vector.tensor_mul(
                    out=pTB[:, 0, 0:P], in0=pTB[:, 0, 0:P], in1=tri01,
                )
                nc.gpsimd.tensor_mul(
                    out=pTB[:, 0, 384:512], in0=pTB[:, 0, 384:512], in1=tri01,
                )
                nc.tensor.matmul(
                    po[:, 256:], lhsT=vv[:, jb + 2, h, :], rhs=pTB[:, 0, 0:256],
                    start=False, stop=False,
                )
                nc.tensor.matmul(
                    po[:, 256:], lhsT=vv[:, jb + 3, h, :], rhs=pTB[:, 0, 256:512],
                    start=False, stop=True,
                )
                # normalize: rows 1:65 divided by row 0 (the ones-col sum).
                # The denom lives on PSUM partition 0, so partition_broadcast
                # can fan out its reciprocal without any staging DMA.  The
                # mul covers the aligned rows 0:65 (row 0 becomes den/den=1,
                # harmless); the DMA then ships rows 1:65 into AT.
                rcp = ep.tile([1, 512], f32r, tag="rcp")
                with nc.allow_low_precision(
                    reason="f32r reciprocal of softmax denom; 2^-11 rel"
                ):
                    nc.vector.reciprocal(out=rcp, in_=po[0:1, :])
                rb = ep.tile([HD + 1, 512], f32r, tag="rb")
                nc.gpsimd.partition_broadcast(rb, rcp)
                ob = ep.tile([HD + 1, 512], f32r, tag="ob")
                meng = nc.vector if h % 2 == 0 else nc.gpsimd
                meng.tensor_mul(out=ob, in0=po, in1=rb)
                deng = nc.sync if h % 2 == 0 or qq == NQ - 1 else nc.gpsimd
                deng.dma_start(
                    out=ATq[qq][bp:bp + HD, fo, :], in_=ob[1:HD + 1, :],
                )
                emit_some(2 * nf)

            def unit_misc(stage):
                # small params on the gpsimd queue + ACT exp-table preload;
                # emitted mid-prologue so the first x loads go first on
                # their queues
                def m():
                    nc.gpsimd.dma_start(out=kbias_sb, in_=kbias_d[:])
                    nc.gpsimd.dma_start(out=bqk_sb, in_=bqk_d[:])
                    nc.gpsimd.dma_start(
                        out=bv_sb,
                        in_=bv_d[:].rearrange(
                            "o (h d) -> o h d", h=HL).to_broadcast([P, HL, HD]),
                    )
                    warm = ep.tile([1, 8], f32r, tag="rcp", name="warm")
                    nc.scalar.activation(out=warm, in_=tri01[0:1, 0:8], func=EXP)

                FQ.append((stage, m))

            # ---- schedule --------------------------------------------------
            # stage-0 prologue (everything quad 0 needs); weight chunks
            # interleave with the x loads/transposes
            unit_Wqk(0, 0)
            unit_T(0, 0)
            unit_Wqk(2, 0)
            unit_T(1, 0)
            unit_Wv(0, 0)
            unit_T(2, 0)
            unit_Wv(1, 0)
            unit_T(3, 0)
            unit_misc(0)
            unit_QK(0, 0, 0)
            unit_QK(2, 0, 0)
            unit_V(0, 0)
            unit_Wqk(1, 0)
            unit_V(1, 0)
            unit_Wqk(3, 0)
            unit_QK(1, 0, 0)
            unit_QK(3, 0, 0)
            flush_stage(0)

            for qq in range(NQ):
                if qq < NQ - 1:
                    s = qq + 1
                    for ti in range(4 * s, 4 * s + 4):
                        unit_T(ti, s)
                    if s == 1:
                        unit_Wproj(0, s)
                    unit_QK(0, s, s)
                    unit_QK(2, s, s)
                    if s == 1:
                        unit_Wproj(1, s)
                    unit_V(2 * s, s)
                    unit_QK(1, s, s)
                    unit_V(2 * s + 1, s)
                    unit_QK(3, s, s)
                if 1 <= qq <= 2:
                    # projection of the previous quad interleaves as filler
                    # into this quad and must flush before quad qq+1 ends
                    unit_PR(qq - 1, qq + 2)
                for h in range(HL):
                    attn(qq, h)
                if qq == NQ - 1:
                    # PR(2) emits after the last quad's heads: its matmuls
                    # keep the PE busy while the final head's normalize
                    # chain completes, so PR(3) then flows without a stall
                    unit_PR(qq - 1, 99, use_act=True)
                if qq < NQ - 1:
                    flush_stage(qq + 1)
            flush_all()
            unit_PR(NQ - 1, 99, use_act=True)
            flush_all()

    return nc


def _get_nc():
    if "nc" not in _cache:
        nc = _build_nc()
        nc.finalize()  # runs the Bacc pass pipeline (reg alloc, wait splitting)
        _cache["nc"] = nc
    return _cache["nc"]


def _make_in_maps(x, attn_mask, W_qkv, b_qkv, W_proj):
    x = np.asarray(x, dtype=np.float32)
    attn_mask = np.asarray(attn_mask)
    W_qkv = np.asarray(W_qkv, dtype=np.float32)
    b_qkv = np.asarray(b_qkv, dtype=np.float32)
    W_proj = np.asarray(W_proj, dtype=np.float32)

    in_maps = []
    for core in range(8):
        b, g = core // 4, core % 4
        s = slice(CL * g, CL * (g + 1))
        wq = W_qkv[:, 0 * C:1 * C][:, s]
        wk = W_qkv[:, 1 * C:2 * C][:, s]
        wv = W_qkv[:, 2 * C:3 * C][:, s]
        bq = b_qkv[0 * C:1 * C][s]
        bk = b_qkv[1 * C:2 * C][s]
        bv = b_qkv[2 * C:3 * C][s]
        bqk = np.concatenate([bq, bk]).reshape(4, P).T  # [128,4], f = fi*128+p
        kbias = (attn_mask[b] != 0).astype(np.float32)  # 0/1 key mask
        in_maps.append({
            "x": np.ascontiguousarray(x[b]),
            "wqk": np.ascontiguousarray(np.concatenate([wq, wk], axis=1)),
            "wv": np.ascontiguousarray(wv),
            "bqk": np.ascontiguousarray(bqk),
            "bv": np.ascontiguousarray(bv.reshape(1, CL)),
            "kbias": np.ascontiguousarray(kbias.reshape(16, P).T),
            "wproj": np.ascontiguousarray(W_proj[s, :]),
        })
    return in_maps


def kernel(x, attn_mask, W_qkv, b_qkv, W_proj, b_proj, _trace=False):
    from concourse.bass_utils import run_bass_kernel_spmd

    nc = _get_nc()
    in_maps = _make_in_maps(x, attn_mask, W_qkv, b_qkv, W_proj)
    res = run_bass_kernel_spmd(nc, in_maps, list(range(8)), trace=_trace)
    outs = res.results

    b_proj = np.asarray(b_proj, dtype=np.float32)
    y = np.empty((2, T, C), dtype=np.float32)
    for b in range(2):
        acc = outs[b * 4]["yT"].T.astype(np.float32).copy()
        for g in range(1, 4):
            acc += outs[b * 4 + g]["yT"].T
        y[b] = acc + b_proj
    if _trace:
        return y, res
    return y
